# revision 1
# baseline (speedup 1.0000x reference)
"""Trainium2 Bass kernel for nn_DynamicFusionModule (moe_routing).

Structure (8 NeuronCores, SPMD):
  Launch A (routing): the 9216 pixels (B=4 x N=2304) are split 8 ways; each
    core runs the SamplingAgent MLP (512->512 silu ->1) on its 1152 pixels in
    fp32r and also emits base = f_ir + f_vis for its slice.
  Host: mask = logits > 0 per batch; top-64 fallback exactly as the reference;
    selected indices are gathered per batch.
  Launch B (experts): one core per (batch, modality) runs the full MixerBlock
    (pre-LN MHA over the selected tokens + pre-LN FFN) on the gathered,
    padded-to-S_PAD token set, channel-major, fp32r matmuls.
  Host: scatter refined = (ri + rv) * mask into the base canvas.

All shapes are hardcoded for the fixed problem instance:
  f_ir, f_vis: [4, 256, 48, 48] fp32.
"""
import math
from contextlib import ExitStack

import numpy as np

import concourse.bass as bass
from concourse import bacc
import concourse.mybir as mybir
import concourse.tile as tile
from concourse.bass_utils import run_bass_kernel_spmd

FP32 = mybir.dt.float32
FP32R = mybir.dt.float32r
ACT = mybir.ActivationFunctionType
ALU = mybir.AluOpType

B = 4
C = 256        # model dim
H = W = 48
N = H * W      # 2304 tokens per batch
HEADS = 4
F = 1024       # FFN hidden
BLK = C // 128  # 2
FB = F // 128   # 8
CIN = 512      # agent input channels
HID = 512      # agent hidden
TPC = (B * N) // 8  # agent tokens per core = 1152
MIN_TOK = 64
S_PAD_DEFAULT = 1472  # >= max selected count (1451 for the fixed seed), mult of 64


def _chunks(total, size):
    out, o = [], 0
    while o < total:
        w = min(size, total - o)
        out.append((o, w))
        o += w
    return out


# ----------------------------------------------------------------------------
# Launch A: agent logits + base canvas
# ----------------------------------------------------------------------------
def build_agent_nc():
    nc = bacc.Bacc("TRN2", target_bir_lowering=False)
    x_d = nc.declare_dram_parameter("x", [CIN, TPC], FP32, isOutput=False)
    aw1t_d = nc.declare_dram_parameter("aw1t", [CIN, HID], FP32, isOutput=False)
    ab1_d = nc.declare_dram_parameter("ab1", [HID], FP32, isOutput=False)
    aw2t_d = nc.declare_dram_parameter("aw2t", [HID], FP32, isOutput=False)
    ab2_d = nc.declare_dram_parameter("ab2", [1], FP32, isOutput=False)
    lg_out = nc.declare_dram_parameter("logits", [TPC], FP32, isOutput=True)
    base_out = nc.declare_dram_parameter("base", [C, TPC], FP32, isOutput=True)

    KO = CIN // 128  # 4
    MO = HID // 128  # 4
    TCH = 384
    NT = TPC // TCH  # 3

    with tile.TileContext(nc) as tc, ExitStack() as ctx:
        sb = ctx.enter_context(tc.tile_pool(name="sb", bufs=1))
        ps = ctx.enter_context(tc.tile_pool(name="ps", bufs=2, space="PSUM"))

        x_f = sb.tile([128, KO, TPC], FP32)
        nc.sync.dma_start(x_f, x_d.rearrange("(ko p) t -> p ko t", p=128))
        w1_sb = sb.tile([128, KO, HID], FP32R)
        nc.gpsimd.dma_start(out=w1_sb,
                            in_=aw1t_d.rearrange("(ko p) m -> p ko m", p=128))
        b1_sb = sb.tile([128, MO], FP32)
        nc.sync.dma_start(b1_sb, ab1_d.rearrange("(mo p) -> p mo", p=128))
        w2_sb = sb.tile([128, MO], FP32R)
        nc.gpsimd.dma_start(out=w2_sb, in_=aw2t_d.rearrange("(mo p) -> p mo", p=128))
        ab2_sb = sb.tile([1, 1], FP32)
        nc.sync.dma_start(ab2_sb, ab2_d.rearrange("(a o) -> a o", a=1))

        x_sb = sb.tile([128, KO, TPC], FP32R)
        nc.vector.tensor_copy(out=x_sb, in_=x_f)

        # base = f_ir + f_vis for this slice (blocks 0,1 + blocks 2,3)
        base_sb = sb.tile([128, 2, TPC], FP32)
        nc.vector.tensor_tensor(base_sb, x_f[:, 0:2], x_f[:, 2:4], ALU.add)
        nc.sync.dma_start(base_out.rearrange("(blk p) t -> p blk t", p=128), base_sb)

        h1_sb = sb.tile([128, MO, TPC], FP32R)
        for mo in range(MO):
            for t in range(NT):
                p = ps.tile([128, TCH], FP32, tag="acc")
                for ko in range(KO):
                    nc.tensor.matmul(
                        p, w1_sb[:, ko, mo * 128:(mo + 1) * 128],
                        x_sb[:, ko, t * TCH:(t + 1) * TCH],
                        start=(ko == 0), stop=(ko == KO - 1))
                nc.scalar.activation(
                    out=h1_sb[:, mo, t * TCH:(t + 1) * TCH], in_=p,
                    func=ACT.Silu, bias=b1_sb[:, mo:mo + 1], scale=1.0)

        lg_sb = sb.tile([1, TPC], FP32)
        for t in range(NT):
            p2 = ps.tile([1, TCH], FP32, tag="acc2")
            for mo in range(MO):
                nc.tensor.matmul(
                    p2, w2_sb[:, mo:mo + 1],
                    h1_sb[:, mo, t * TCH:(t + 1) * TCH],
                    start=(mo == 0), stop=(mo == MO - 1))
            nc.scalar.activation(out=lg_sb[:, t * TCH:(t + 1) * TCH], in_=p2,
                                 func=ACT.Identity, bias=ab2_sb)
        nc.sync.dma_start(lg_out.rearrange("(o t) -> o t", o=1), lg_sb)
    return nc


# ----------------------------------------------------------------------------
# Launch B: MixerBlock on S gathered tokens (see dev notes in docstring)
# ----------------------------------------------------------------------------
def build_mixer_nc(S: int):
    assert S % 64 == 0
    KT = _chunks(S, 128)
    QC = _chunks(S, 512)
    NKT = len(KT)
    KBIAS_LEN = 128 * NKT

    nc = bacc.Bacc("TRN2", target_bir_lowering=False)
    xg_d = nc.declare_dram_parameter("xg", [C, S], FP32, isOutput=False)
    kb_d = nc.declare_dram_parameter("kbias", [KBIAS_LEN], FP32, isOutput=False)
    lng_d = nc.declare_dram_parameter("lng", [C], FP32, isOutput=False)
    lnb_d = nc.declare_dram_parameter("lnb", [C], FP32, isOutput=False)
    wqkT_d = nc.declare_dram_parameter("wqkT", [C, 512], FP32, isOutput=False)
    bqk_d = nc.declare_dram_parameter("bqk", [512], FP32, isOutput=False)
    wvT_d = nc.declare_dram_parameter("wvT", [C, C], FP32, isOutput=False)
    bv_d = nc.declare_dram_parameter("bv", [C], FP32, isOutput=False)
    woT_d = nc.declare_dram_parameter("woT", [C, C], FP32, isOutput=False)
    bo_d = nc.declare_dram_parameter("bo", [C], FP32, isOutput=False)
    w1T_d = nc.declare_dram_parameter("w1T", [C, F], FP32, isOutput=False)
    b1_d = nc.declare_dram_parameter("b1", [F], FP32, isOutput=False)
    w2T_d = nc.declare_dram_parameter("w2T", [F, C], FP32, isOutput=False)
    b2_d = nc.declare_dram_parameter("b2", [C], FP32, isOutput=False)
    yg_d = nc.declare_dram_parameter("yg", [C, S], FP32, isOutput=True)

    with tile.TileContext(nc) as tc, ExitStack() as ctx:
        sb = ctx.enter_context(tc.tile_pool(name="sb", bufs=1))
        scr = ctx.enter_context(tc.tile_pool(name="scr", bufs=2))
        scr1 = ctx.enter_context(tc.tile_pool(name="scr1", bufs=1))
        expp = ctx.enter_context(tc.tile_pool(name="expp", bufs=2))
        psmm = ctx.enter_context(tc.tile_pool(name="psmm", bufs=2, space="PSUM"))
        pssc = ctx.enter_context(tc.tile_pool(name="pssc", bufs=2, space="PSUM"))
        pso = ctx.enter_context(tc.tile_pool(name="pso", bufs=2, space="PSUM"))
        psrep = ctx.enter_context(tc.tile_pool(name="psrep", bufs=2, space="PSUM"))

        x_sb = sb.tile([128, BLK, S], FP32, tag="xy")
        nc.sync.dma_start(x_sb, xg_d.rearrange("(blk p) t -> p blk t", p=128))
        kb_sb = sb.tile([128, NKT], FP32)
        nc.sync.dma_start(kb_sb, kb_d.rearrange("(kt p) -> p kt", p=128))
        lng_sb = sb.tile([128, BLK], FP32)
        nc.sync.dma_start(lng_sb, lng_d.rearrange("(blk p) -> p blk", p=128))
        lnb_sb = sb.tile([128, BLK], FP32)
        nc.sync.dma_start(lnb_sb, lnb_d.rearrange("(blk p) -> p blk", p=128))
        bqk_sb = sb.tile([128, 4], FP32)
        nc.sync.dma_start(bqk_sb, bqk_d.rearrange("(m p) -> p m", p=128))
        bo_sb = sb.tile([128, BLK], FP32)
        nc.sync.dma_start(bo_sb, bo_d.rearrange("(m p) -> p m", p=128))
        b1_sb = sb.tile([128, FB], FP32)
        nc.sync.dma_start(b1_sb, b1_d.rearrange("(m p) -> p m", p=128))
        b2_sb = sb.tile([128, BLK], FP32)
        nc.sync.dma_start(b2_sb, b2_d.rearrange("(m p) -> p m", p=128))
        bv_ap = bv_d[:]
        bv_bc_src = bass.AP(tensor=bv_ap.tensor, offset=bv_ap.offset,
                            ap=[[0, 128]] + [list(p) for p in bv_ap.ap])
        bv_sb = sb.tile([128, C], FP32)
        nc.gpsimd.dma_start(out=bv_sb, in_=bv_bc_src)

        def load_w(dram, cols, kblocks, nm):
            t_r = sb.tile([128, kblocks, cols], FP32R, name=nm)
            nc.gpsimd.dma_start(out=t_r,
                                in_=dram.rearrange("(kb p) m -> p kb m", p=128))
            return t_r

        wqkT_sb = load_w(wqkT_d, 512, BLK, "wqkT_sb")
        wvT_sb = load_w(wvT_d, C, BLK, "wvT_sb")
        woT_sb = load_w(woT_d, C, BLK, "woT_sb")
        w1T_sb = load_w(w1T_d, F, BLK, "w1T_sb")
        w2T_sb = load_w(w2T_d, C, FB, "w2T_sb")

        ones_f = sb.tile([128, 128], FP32)
        nc.vector.memset(ones_f, 1.0)
        ones_m = sb.tile([1, 128], FP32R)   # lhsT for partition-replicate
        nc.vector.tensor_copy(out=ones_m, in_=ones_f[0:1])
        ones_k = sb.tile([128, 1], FP32R)   # lhsT for channel-sum
        nc.vector.tensor_copy(out=ones_k, in_=ones_f[:, 0:1])
        eps_sb = sb.tile([1, 1], FP32)
        nc.vector.memset(eps_sb, 1e-5)

        def layernorm(x_in, xn_out, uid):
            mean_r = sb.tile([1, S], FP32R, name=f"mean_{uid}")
            rstd_r = sb.tile([1, S], FP32R, name=f"rstd_{uid}")
            for (qo, qw) in QC:
                xr_c = scr.tile([128, BLK, 512], FP32R, tag="xr_c")
                nc.vector.tensor_copy(out=xr_c[:, :, :qw], in_=x_in[:, :, qo:qo + qw])
                xsq_c = scr.tile([128, BLK, 512], FP32R, tag="xsq_c")
                nc.scalar.activation(out=xsq_c[:, :, :qw], in_=x_in[:, :, qo:qo + qw],
                                     func=ACT.Square)
                ps_s = psrep.tile([128, 512], FP32, tag="rep")
                ps_q = psrep.tile([128, 512], FP32, tag="rep")
                for blk in range(BLK):
                    nc.tensor.matmul(ps_s[0:1, :qw], ones_k, xr_c[:, blk, :qw],
                                     start=(blk == 0), stop=(blk == BLK - 1))
                for blk in range(BLK):
                    nc.tensor.matmul(ps_q[0:1, :qw], ones_k, xsq_c[:, blk, :qw],
                                     start=(blk == 0), stop=(blk == BLK - 1))
                nc.scalar.mul(out=mean_r[:, qo:qo + qw], in_=ps_s[0:1, :qw],
                              mul=1.0 / C)
                m2 = scr.tile([1, 512], FP32, tag="s512")
                nc.vector.tensor_tensor(m2[:, :qw], mean_r[:, qo:qo + qw],
                                        mean_r[:, qo:qo + qw], ALU.mult)
                var_c = scr.tile([1, 512], FP32, tag="s512")
                nc.scalar.mul(out=var_c[:, :qw], in_=ps_q[0:1, :qw], mul=1.0 / C)
                nc.vector.tensor_tensor(var_c[:, :qw], var_c[:, :qw], m2[:, :qw],
                                        ALU.subtract)
                sd_c = scr.tile([1, 512], FP32, tag="s512")
                nc.scalar.activation(out=sd_c[:, :qw], in_=var_c[:, :qw],
                                     func=ACT.Sqrt, bias=eps_sb)
                rec_f = scr.tile([1, 512], FP32, tag="s512")
                nc.vector.reciprocal(out=rec_f[:, :qw], in_=sd_c[:, :qw])
                nc.vector.tensor_copy(out=rstd_r[:, qo:qo + qw], in_=rec_f[:, :qw])
            for (qo, qw) in QC:
                rep_m = psrep.tile([128, 512], FP32, tag="rep")
                nc.tensor.matmul(rep_m[:, :qw], ones_m, mean_r[:, qo:qo + qw],
                                 start=True, stop=True)
                rep_s = psrep.tile([128, 512], FP32, tag="rep")
                nc.tensor.matmul(rep_s[:, :qw], ones_m, rstd_r[:, qo:qo + qw],
                                 start=True, stop=True)
                for blk in range(BLK):
                    t = scr.tile([128, 512], FP32, tag="t512")
                    nc.vector.tensor_tensor(t[:, :qw], x_in[:, blk, qo:qo + qw],
                                            rep_m[:, :qw], ALU.subtract)
                    nc.vector.tensor_tensor(t[:, :qw], t[:, :qw], rep_s[:, :qw],
                                            ALU.mult)
                    nc.vector.tensor_scalar(
                        out=xn_out[:, blk, qo:qo + qw], in0=t[:, :qw],
                        scalar1=lng_sb[:, blk:blk + 1], scalar2=lnb_sb[:, blk:blk + 1],
                        op0=ALU.mult, op1=ALU.add)

        # ---- attention ----
        xn_sb = sb.tile([128, BLK, S], FP32R, name="xn_sb")
        layernorm(x_sb, xn_sb, "ln1")

        qk_sb = sb.tile([128, 4, S], FP32R)
        for mt in range(4):
            for (qo, qw) in QC:
                p = psmm.tile([128, 512], FP32, tag="mm")
                for blk in range(BLK):
                    nc.tensor.matmul(p[:, :qw],
                                     wqkT_sb[:, blk, mt * 128:(mt + 1) * 128],
                                     xn_sb[:, blk, qo:qo + qw],
                                     start=(blk == 0), stop=(blk == BLK - 1))
                nc.scalar.activation(out=qk_sb[:, mt, qo:qo + qw], in_=p[:, :qw],
                                     func=ACT.Identity, bias=bqk_sb[:, mt:mt + 1])

        v_sb = sb.tile([128, NKT, HEADS, 65], FP32R)
        nc.vector.tensor_copy(
            out=v_sb[:, :, :, 64:65],
            in_=ones_f[:, 0:1, None, None].to_broadcast([128, NKT, HEADS, 1]))
        for kt, (ko, kw) in enumerate(KT):
            p = psmm.tile([128, 512], FP32, tag="mm")
            for blk in range(BLK):
                nc.tensor.matmul(p[:kw, :C], xn_sb[:, blk, ko:ko + kw],
                                 wvT_sb[:, blk, :],
                                 start=(blk == 0), stop=(blk == BLK - 1))
            nc.vector.tensor_tensor(
                v_sb[:kw, kt, :, 0:64],
                p[:kw, :C].rearrange("p (h d) -> p h d", h=HEADS),
                bv_sb[:kw].rearrange("p (h d) -> p h d", h=HEADS),
                ALU.add)

        attn_sb = sb.tile([128, BLK, S], FP32R, tag="attn")
        for (qo, qw) in QC:
            for h in range(HEADS):
                pr = slice((h % 2) * 64, (h % 2) * 64 + 64)
                qblk = h // 2
                exp_c = expp.tile([128, NKT, 512], FP32R, tag="exp")
                for kt, (ko, kw) in enumerate(KT):
                    ps_sc = pssc.tile([128, 512], FP32, tag="sc")
                    nc.tensor.matmul(ps_sc[:kw, :qw],
                                     qk_sb[pr, 2 + qblk, ko:ko + kw],
                                     qk_sb[pr, qblk, qo:qo + qw],
                                     start=True, stop=True)
                    nc.scalar.activation(out=exp_c[:kw, kt, :qw], in_=ps_sc[:kw, :qw],
                                         func=ACT.Exp,
                                         bias=kb_sb[:kw, kt:kt + 1], scale=0.125)
                ps_o = pso.tile([65, 512], FP32, tag="o")
                for kt, (ko, kw) in enumerate(KT):
                    nc.tensor.matmul(ps_o[:, :qw], v_sb[:kw, kt, h, :],
                                     exp_c[:kw, kt, :qw],
                                     start=(kt == 0), stop=(kt == NKT - 1))
                rec_f = scr.tile([1, 512], FP32, tag="s512")
                nc.vector.reciprocal(out=rec_f[:, :qw], in_=ps_o[64:65, :qw])
                recd = scr1.tile([1, 512], FP32R, tag="recd")
                nc.vector.tensor_copy(out=recd[:, :qw], in_=rec_f[:, :qw])
                rep_d = psrep.tile([128, 512], FP32, tag="rep")
                nc.tensor.matmul(rep_d[0:64, :qw], ones_m[:, 0:64], recd[:, :qw],
                                 start=True, stop=True)
                onum = scr1.tile([64, 512], FP32, tag="onum")
                nc.scalar.activation(out=onum[:, :qw], in_=ps_o[0:64, :qw],
                                     func=ACT.Copy)
                nc.vector.tensor_tensor(attn_sb[pr, qblk, qo:qo + qw],
                                        onum[:, :qw], rep_d[0:64, :qw], ALU.mult)

        x2_sb = sb.tile([128, BLK, S], FP32)
        for mt in range(BLK):
            for (qo, qw) in QC:
                p = psmm.tile([128, 512], FP32, tag="mm")
                for blk in range(BLK):
                    nc.tensor.matmul(p[:, :qw],
                                     woT_sb[:, blk, mt * 128:(mt + 1) * 128],
                                     attn_sb[:, blk, qo:qo + qw],
                                     start=(blk == 0), stop=(blk == BLK - 1))
                t = scr.tile([128, 512], FP32, tag="t512")
                nc.scalar.activation(out=t[:, :qw], in_=p[:, :qw],
                                     func=ACT.Identity, bias=bo_sb[:, mt:mt + 1])
                nc.vector.tensor_tensor(x2_sb[:, mt, qo:qo + qw], t[:, :qw],
                                        x_sb[:, mt, qo:qo + qw], ALU.add)

        # ---- FFN ----
        xn2_sb = sb.tile([128, BLK, S], FP32R, tag="attn")  # reuse attn buffer
        layernorm(x2_sb, xn2_sb, "ln2")

        y_sb = sb.tile([128, BLK, S], FP32, tag="xy")       # reuse x buffer
        for (qo, qw) in QC:
            h1_full = expp.tile([128, NKT, 512], FP32R, tag="exp", name="h1_full")
            h1_c = h1_full[:, :FB, :]
            for mt in range(FB):
                p = psmm.tile([128, 512], FP32, tag="mm")
                for blk in range(BLK):
                    nc.tensor.matmul(p[:, :qw],
                                     w1T_sb[:, blk, mt * 128:(mt + 1) * 128],
                                     xn2_sb[:, blk, qo:qo + qw],
                                     start=(blk == 0), stop=(blk == BLK - 1))
                nc.scalar.activation(out=h1_c[:, mt, :qw], in_=p[:, :qw],
                                     func=ACT.Gelu, bias=b1_sb[:, mt:mt + 1])
            for mt in range(BLK):
                p = psmm.tile([128, 512], FP32, tag="mm")
                for kb in range(FB):
                    nc.tensor.matmul(p[:, :qw],
                                     w2T_sb[:, kb, mt * 128:(mt + 1) * 128],
                                     h1_c[:, kb, :qw],
                                     start=(kb == 0), stop=(kb == FB - 1))
                t = scr.tile([128, 512], FP32, tag="t512")
                nc.scalar.activation(out=t[:, :qw], in_=p[:, :qw],
                                     func=ACT.Identity, bias=b2_sb[:, mt:mt + 1])
                nc.vector.tensor_tensor(y_sb[:, mt, qo:qo + qw], t[:, :qw],
                                        x2_sb[:, mt, qo:qo + qw], ALU.add)

        nc.sync.dma_start(yg_d.rearrange("(blk p) t -> p blk t", p=128), y_sb)
    return nc


# ----------------------------------------------------------------------------
# Host orchestration
# ----------------------------------------------------------------------------
_CACHE = {}


def _get_agent_nc():
    if "agent" not in _CACHE:
        nc = build_agent_nc()
        nc.finalize()
        _CACHE["agent"] = nc
    return _CACHE["agent"]


def _get_mixer_nc(S):
    key = ("mixer", S)
    if key not in _CACHE:
        nc = build_mixer_nc(S)
        nc.finalize()
        _CACHE[key] = nc
    return _CACHE[key]


def kernel(f_ir, f_vis, aw1, ab1, aw2, ab2,
           ir_lng, ir_lnb, ir_wqkv, ir_bqkv, ir_wo, ir_bo, ir_w1, ir_b1,
           ir_w2, ir_b2,
           vis_lng, vis_lnb, vis_wqkv, vis_bqkv, vis_wo, vis_bo, vis_w1,
           vis_b1, vis_w2, vis_b2):
    f_ir = np.ascontiguousarray(f_ir, np.float32)
    f_vis = np.ascontiguousarray(f_vis, np.float32)

    # ---- launch A: routing logits + base canvas, token-parallel over 8 cores
    fir_n = f_ir.reshape(B, C, N)
    fvis_n = f_vis.reshape(B, C, N)
    X = np.concatenate([fir_n, fvis_n], axis=1)            # [B, 512, N]
    Xf = X.reshape(B, CIN, 2, TPC).transpose(0, 2, 1, 3).reshape(8, CIN, TPC)
    aw1t = np.ascontiguousarray(aw1.T, np.float32)
    aw2t = np.ascontiguousarray(aw2[0], np.float32)
    ab1 = np.ascontiguousarray(ab1, np.float32)
    ab2 = np.ascontiguousarray(ab2, np.float32)

    nc_a = _get_agent_nc()
    in_maps = [dict(x=np.ascontiguousarray(Xf[i]), aw1t=aw1t, ab1=ab1,
                    aw2t=aw2t, ab2=ab2) for i in range(8)]
    ra = run_bass_kernel_spmd(nc_a, in_maps, list(range(8)))
    logits = np.stack([ra.results[i]["logits"] for i in range(8)])
    logits = logits.reshape(B, 2 * TPC)                    # [B, N]
    base = np.stack([ra.results[i]["base"] for i in range(8)])
    base = base.reshape(B, 2, C, TPC).transpose(0, 2, 1, 3).reshape(B, C, N)

    # ---- host routing decision (reference semantics)
    mask = (logits > 0)
    counts = mask.sum(1)
    sel = np.empty_like(mask)
    for b in range(B):
        if counts[b] < MIN_TOK:
            top = np.argsort(-logits[b], kind="stable")[:MIN_TOK]
            s = np.zeros(N, bool)
            s[top] = True
            sel[b] = s
        else:
            sel[b] = mask[b]
    idxs = [np.where(sel[b])[0] for b in range(B)]
    s_max = max(len(i) for i in idxs)
    S = S_PAD_DEFAULT if s_max <= S_PAD_DEFAULT else ((s_max + 63) // 64) * 64
    NKT = len(_chunks(S, 128))

    # ---- launch B: one (batch, modality) mixer per core
    nc_b = _get_mixer_nc(S)
    in_maps_b = []
    metas = []
    for b in range(B):
        idx = idxs[b]
        Sb = len(idx)
        kbias = np.full((128 * NKT,), np.float32(-1e9), np.float32)
        kbias[:Sb] = 0.0
        for mod, fm, pfx in (("ir", fir_n[b], "ir"), ("vis", fvis_n[b], "vis")):
            params = {
                "lng": ir_lng if pfx == "ir" else vis_lng,
                "lnb": ir_lnb if pfx == "ir" else vis_lnb,
                "wqkv": ir_wqkv if pfx == "ir" else vis_wqkv,
                "bqkv": ir_bqkv if pfx == "ir" else vis_bqkv,
                "wo": ir_wo if pfx == "ir" else vis_wo,
                "bo": ir_bo if pfx == "ir" else vis_bo,
                "w1": ir_w1 if pfx == "ir" else vis_w1,
                "b1": ir_b1 if pfx == "ir" else vis_b1,
                "w2": ir_w2 if pfx == "ir" else vis_w2,
                "b2": ir_b2 if pfx == "ir" else vis_b2,
            }
            xg = np.zeros((C, S), np.float32)
            xg[:, :Sb] = fm[:, idx]
            wqkv = np.asarray(params["wqkv"], np.float32)
            im = dict(
                xg=xg, kbias=kbias,
                lng=np.ascontiguousarray(params["lng"], np.float32),
                lnb=np.ascontiguousarray(params["lnb"], np.float32),
                wqkT=np.ascontiguousarray(wqkv[:512].T),
                bqk=np.ascontiguousarray(params["bqkv"][:512], np.float32),
                wvT=np.ascontiguousarray(wqkv[512:].T),
                bv=np.ascontiguousarray(params["bqkv"][512:], np.float32),
                woT=np.ascontiguousarray(np.asarray(params["wo"], np.float32).T),
                bo=np.ascontiguousarray(params["bo"], np.float32),
                w1T=np.ascontiguousarray(np.asarray(params["w1"], np.float32).T),
                b1=np.ascontiguousarray(params["b1"], np.float32),
                w2T=np.ascontiguousarray(np.asarray(params["w2"], np.float32).T),
                b2=np.ascontiguousarray(params["b2"], np.float32),
            )
            in_maps_b.append(im)
            metas.append((b, mod, idx))
    rb = run_bass_kernel_spmd(nc_b, in_maps_b, list(range(8)))

    # ---- host scatter-combine
    out = base  # [B, C, N]; refined overwrites selected positions
    for ci in range(0, 8, 2):
        b, _, idx = metas[ci]
        Sb = len(idx)
        ri = rb.results[ci]["yg"][:, :Sb]
        rv = rb.results[ci + 1]["yg"][:, :Sb]
        refined = (ri + rv) * mask[b, idx].astype(np.float32)[None, :]
        out[b][:, idx] = refined
    return out.reshape(B, C, H, W)


# revision 4
# speedup vs baseline: 1.0260x; 1.0260x over previous
"""Trainium2 Bass kernel for nn_DynamicFusionModule (moe_routing).

Structure (8 NeuronCores, SPMD):
  Launch A (routing): the 9216 pixels (B=4 x N=2304) are split 8 ways; each
    core runs the SamplingAgent MLP (512->512 silu ->1) on its 1152 pixels in
    fp32r and also emits base = f_ir + f_vis for its slice.
  Host: mask = logits > 0 per batch; top-64 fallback exactly as the reference;
    selected indices are gathered per batch.
  Launch B (experts): one core per (batch, modality) runs the full MixerBlock
    (pre-LN MHA over the selected tokens + pre-LN FFN) on the gathered,
    padded-to-S_PAD token set, channel-major, fp32r matmuls.
  Host: scatter refined = (ri + rv) * mask into the base canvas.

All shapes are hardcoded for the fixed problem instance:
  f_ir, f_vis: [4, 256, 48, 48] fp32.
"""
import math
from contextlib import ExitStack

import numpy as np

import concourse.bass as bass
from concourse import bacc
import concourse.mybir as mybir
import concourse.tile as tile
from concourse.bass_utils import run_bass_kernel_spmd

FP32 = mybir.dt.float32
FP32R = mybir.dt.float32r
ACT = mybir.ActivationFunctionType
ALU = mybir.AluOpType

B = 4
C = 256        # model dim
H = W = 48
N = H * W      # 2304 tokens per batch
HEADS = 4
F = 1024       # FFN hidden
BLK = C // 128  # 2
FB = F // 128   # 8
CIN = 512      # agent input channels
HID = 512      # agent hidden
TPC = (B * N) // 8  # agent tokens per core = 1152
MIN_TOK = 64
S_PAD_DEFAULT = 1472  # >= max selected count (1451 for the fixed seed), mult of 64


def _chunks(total, size):
    out, o = [], 0
    while o < total:
        w = min(size, total - o)
        out.append((o, w))
        o += w
    return out


# ----------------------------------------------------------------------------
# Launch A: agent logits + base canvas
# ----------------------------------------------------------------------------
def build_agent_nc():
    nc = bacc.Bacc("TRN2", target_bir_lowering=False)
    x_d = nc.declare_dram_parameter("x", [CIN, TPC], FP32, isOutput=False)
    aw1t_d = nc.declare_dram_parameter("aw1t", [CIN, HID], FP32, isOutput=False)
    ab1_d = nc.declare_dram_parameter("ab1", [HID], FP32, isOutput=False)
    aw2t_d = nc.declare_dram_parameter("aw2t", [HID], FP32, isOutput=False)
    ab2_d = nc.declare_dram_parameter("ab2", [1], FP32, isOutput=False)
    lg_out = nc.declare_dram_parameter("logits", [TPC], FP32, isOutput=True)
    base_out = nc.declare_dram_parameter("base", [C, TPC], FP32, isOutput=True)

    KO = CIN // 128  # 4
    MO = HID // 128  # 4
    TCH = 384
    NT = TPC // TCH  # 3

    with tile.TileContext(nc) as tc, ExitStack() as ctx:
        sb = ctx.enter_context(tc.tile_pool(name="sb", bufs=1))
        xin = ctx.enter_context(tc.tile_pool(name="xin", bufs=2))
        ps = ctx.enter_context(tc.tile_pool(name="ps", bufs=2, space="PSUM"))

        w1_sb = sb.tile([128, KO, HID], FP32R)
        nc.gpsimd.dma_start(out=w1_sb,
                            in_=aw1t_d.rearrange("(ko p) m -> p ko m", p=128))
        b1_sb = sb.tile([128, MO], FP32)
        nc.sync.dma_start(b1_sb, ab1_d.rearrange("(mo p) -> p mo", p=128))
        w2_sb = sb.tile([128, MO], FP32R)
        nc.gpsimd.dma_start(out=w2_sb, in_=aw2t_d.rearrange("(mo p) -> p mo", p=128))
        ab2_sb = sb.tile([1, 1], FP32)
        nc.sync.dma_start(ab2_sb, ab2_d.rearrange("(a o) -> a o", a=1))

        x_r = x_d.rearrange("(ko p) t -> p ko t", p=128)
        base_r = base_out.rearrange("(blk p) t -> p blk t", p=128)
        lg_sb = sb.tile([1, TPC], FP32)
        # token-chunk pipeline: DMA(t+1) overlaps compute(t)
        for t in range(NT):
            tsl = slice(t * TCH, (t + 1) * TCH)
            x_f = xin.tile([128, KO, TCH], FP32, tag="x_f")
            nc.sync.dma_start(x_f, x_r[:, :, tsl])
            x_sb = xin.tile([128, KO, TCH], FP32R, tag="x_r")
            nc.vector.tensor_copy(out=x_sb, in_=x_f)
            # base = f_ir + f_vis for this slice (blocks 0,1 + blocks 2,3)
            base_sb = xin.tile([128, 2, TCH], FP32, tag="base")
            nc.vector.tensor_tensor(base_sb, x_f[:, 0:2], x_f[:, 2:4], ALU.add)
            nc.sync.dma_start(base_r[:, :, tsl], base_sb)

            h1_sb = xin.tile([128, MO, TCH], FP32R, tag="h1")
            for mo in range(MO):
                p = ps.tile([128, TCH], FP32, tag="acc")
                for ko in range(KO):
                    nc.tensor.matmul(
                        p, w1_sb[:, ko, mo * 128:(mo + 1) * 128],
                        x_sb[:, ko, :],
                        start=(ko == 0), stop=(ko == KO - 1))
                nc.scalar.activation(
                    out=h1_sb[:, mo, :], in_=p,
                    func=ACT.Silu, bias=b1_sb[:, mo:mo + 1], scale=1.0)

            p2 = ps.tile([1, TCH], FP32, tag="acc2")
            for mo in range(MO):
                nc.tensor.matmul(
                    p2, w2_sb[:, mo:mo + 1], h1_sb[:, mo, :],
                    start=(mo == 0), stop=(mo == MO - 1))
            nc.scalar.activation(out=lg_sb[:, tsl], in_=p2,
                                 func=ACT.Identity, bias=ab2_sb)
        nc.sync.dma_start(lg_out.rearrange("(o t) -> o t", o=1), lg_sb)
    return nc


# ----------------------------------------------------------------------------
# Launch B: MixerBlock on S gathered tokens (see dev notes in docstring)
# ----------------------------------------------------------------------------
def build_mixer_nc(S: int):
    KT = _chunks(S, 128)
    QC = _chunks(S, 512)
    NKT = len(KT)
    KBIAS_LEN = 128 * NKT

    nc = bacc.Bacc("TRN2", target_bir_lowering=False)
    xg_d = nc.declare_dram_parameter("xg", [C, S], FP32, isOutput=False)
    kb_d = nc.declare_dram_parameter("kbias", [KBIAS_LEN], FP32, isOutput=False)
    lng_d = nc.declare_dram_parameter("lng", [C], FP32, isOutput=False)
    lnb_d = nc.declare_dram_parameter("lnb", [C], FP32, isOutput=False)
    wqkT_d = nc.declare_dram_parameter("wqkT", [C, 512], FP32, isOutput=False)
    bqk_d = nc.declare_dram_parameter("bqk", [512], FP32, isOutput=False)
    wvT_d = nc.declare_dram_parameter("wvT", [C, C], FP32, isOutput=False)
    bv_d = nc.declare_dram_parameter("bv", [C], FP32, isOutput=False)
    woT_d = nc.declare_dram_parameter("woT", [C, C], FP32, isOutput=False)
    bo_d = nc.declare_dram_parameter("bo", [C], FP32, isOutput=False)
    w1T_d = nc.declare_dram_parameter("w1T", [C, F], FP32, isOutput=False)
    b1_d = nc.declare_dram_parameter("b1", [F], FP32, isOutput=False)
    w2T_d = nc.declare_dram_parameter("w2T", [F, C], FP32, isOutput=False)
    b2_d = nc.declare_dram_parameter("b2", [C], FP32, isOutput=False)
    yg_d = nc.declare_dram_parameter("yg", [C, S], FP32, isOutput=True)

    with tile.TileContext(nc) as tc, ExitStack() as ctx:
        sb = ctx.enter_context(tc.tile_pool(name="sb", bufs=1))
        scr = ctx.enter_context(tc.tile_pool(name="scr", bufs=2))
        scr1 = ctx.enter_context(tc.tile_pool(name="scr1", bufs=1))
        expp = ctx.enter_context(tc.tile_pool(name="expp", bufs=2))
        psmm = ctx.enter_context(tc.tile_pool(name="psmm", bufs=2, space="PSUM"))
        pssc = ctx.enter_context(tc.tile_pool(name="pssc", bufs=2, space="PSUM"))
        pso = ctx.enter_context(tc.tile_pool(name="pso", bufs=2, space="PSUM"))
        psrep = ctx.enter_context(tc.tile_pool(name="psrep", bufs=2, space="PSUM"))

        x_sb = sb.tile([128, BLK, S], FP32, tag="xy")
        nc.sync.dma_start(x_sb, xg_d.rearrange("(blk p) t -> p blk t", p=128))
        kb_sb = sb.tile([128, NKT], FP32)
        nc.sync.dma_start(kb_sb, kb_d.rearrange("(kt p) -> p kt", p=128))
        lng_sb = sb.tile([128, BLK], FP32)
        nc.sync.dma_start(lng_sb, lng_d.rearrange("(blk p) -> p blk", p=128))
        lnb_sb = sb.tile([128, BLK], FP32)
        nc.sync.dma_start(lnb_sb, lnb_d.rearrange("(blk p) -> p blk", p=128))
        bqk_sb = sb.tile([128, 4], FP32)
        nc.sync.dma_start(bqk_sb, bqk_d.rearrange("(m p) -> p m", p=128))
        bo_sb = sb.tile([128, BLK], FP32)
        nc.sync.dma_start(bo_sb, bo_d.rearrange("(m p) -> p m", p=128))
        b1_sb = sb.tile([128, FB], FP32)
        nc.sync.dma_start(b1_sb, b1_d.rearrange("(m p) -> p m", p=128))
        b2_sb = sb.tile([128, BLK], FP32)
        nc.sync.dma_start(b2_sb, b2_d.rearrange("(m p) -> p m", p=128))
        bv_ap = bv_d[:]
        bv_bc_src = bass.AP(tensor=bv_ap.tensor, offset=bv_ap.offset,
                            ap=[[0, 128]] + [list(p) for p in bv_ap.ap])
        bv_sb = sb.tile([128, C], FP32)
        nc.gpsimd.dma_start(out=bv_sb, in_=bv_bc_src)

        def load_w(dram, cols, kblocks, nm):
            t_r = sb.tile([128, kblocks, cols], FP32R, name=nm)
            nc.gpsimd.dma_start(out=t_r,
                                in_=dram.rearrange("(kb p) m -> p kb m", p=128))
            return t_r

        wqkT_sb = load_w(wqkT_d, 512, BLK, "wqkT_sb")
        wvT_sb = load_w(wvT_d, C, BLK, "wvT_sb")
        woT_sb = load_w(woT_d, C, BLK, "woT_sb")
        w1T_sb = load_w(w1T_d, F, BLK, "w1T_sb")
        w2T_sb = load_w(w2T_d, C, FB, "w2T_sb")

        ones_f = sb.tile([128, 128], FP32)
        nc.vector.memset(ones_f, 1.0)
        ones_m = sb.tile([1, 128], FP32R)   # lhsT for partition-replicate
        nc.vector.tensor_copy(out=ones_m, in_=ones_f[0:1])
        ones_k = sb.tile([128, 1], FP32R)   # lhsT for channel-sum
        nc.vector.tensor_copy(out=ones_k, in_=ones_f[:, 0:1])
        eps_sb = sb.tile([1, 1], FP32)
        nc.vector.memset(eps_sb, 1e-5)

        def layernorm(x_in, xn_out, uid):
            mean_r = sb.tile([1, S], FP32R, name=f"mean_{uid}")
            rstd_r = sb.tile([1, S], FP32R, name=f"rstd_{uid}")
            for (qo, qw) in QC:
                xr_c = scr.tile([128, BLK, 512], FP32R, tag="xr_c")
                nc.vector.tensor_copy(out=xr_c[:, :, :qw], in_=x_in[:, :, qo:qo + qw])
                xsq_c = scr.tile([128, BLK, 512], FP32R, tag="xsq_c")
                nc.scalar.activation(out=xsq_c[:, :, :qw], in_=x_in[:, :, qo:qo + qw],
                                     func=ACT.Square)
                ps_s = psrep.tile([128, 512], FP32, tag="rep")
                ps_q = psrep.tile([128, 512], FP32, tag="rep")
                for blk in range(BLK):
                    nc.tensor.matmul(ps_s[0:1, :qw], ones_k, xr_c[:, blk, :qw],
                                     start=(blk == 0), stop=(blk == BLK - 1))
                for blk in range(BLK):
                    nc.tensor.matmul(ps_q[0:1, :qw], ones_k, xsq_c[:, blk, :qw],
                                     start=(blk == 0), stop=(blk == BLK - 1))
                nc.scalar.mul(out=mean_r[:, qo:qo + qw], in_=ps_s[0:1, :qw],
                              mul=1.0 / C)
                m2 = scr.tile([1, 512], FP32, tag="s512")
                nc.vector.tensor_tensor(m2[:, :qw], mean_r[:, qo:qo + qw],
                                        mean_r[:, qo:qo + qw], ALU.mult)
                var_c = scr.tile([1, 512], FP32, tag="s512")
                nc.scalar.mul(out=var_c[:, :qw], in_=ps_q[0:1, :qw], mul=1.0 / C)
                nc.vector.tensor_tensor(var_c[:, :qw], var_c[:, :qw], m2[:, :qw],
                                        ALU.subtract)
                sd_c = scr.tile([1, 512], FP32, tag="s512")
                nc.scalar.activation(out=sd_c[:, :qw], in_=var_c[:, :qw],
                                     func=ACT.Sqrt, bias=eps_sb)
                rec_f = scr.tile([1, 512], FP32, tag="s512")
                nc.vector.reciprocal(out=rec_f[:, :qw], in_=sd_c[:, :qw])
                nc.vector.tensor_copy(out=rstd_r[:, qo:qo + qw], in_=rec_f[:, :qw])
            for (qo, qw) in QC:
                rep_m = psrep.tile([128, 512], FP32, tag="rep")
                nc.tensor.matmul(rep_m[:, :qw], ones_m, mean_r[:, qo:qo + qw],
                                 start=True, stop=True)
                rep_s = psrep.tile([128, 512], FP32, tag="rep")
                nc.tensor.matmul(rep_s[:, :qw], ones_m, rstd_r[:, qo:qo + qw],
                                 start=True, stop=True)
                for blk in range(BLK):
                    t = scr.tile([128, 512], FP32, tag="t512")
                    nc.vector.tensor_tensor(t[:, :qw], x_in[:, blk, qo:qo + qw],
                                            rep_m[:, :qw], ALU.subtract)
                    nc.vector.tensor_tensor(t[:, :qw], t[:, :qw], rep_s[:, :qw],
                                            ALU.mult)
                    nc.vector.tensor_scalar(
                        out=xn_out[:, blk, qo:qo + qw], in0=t[:, :qw],
                        scalar1=lng_sb[:, blk:blk + 1], scalar2=lnb_sb[:, blk:blk + 1],
                        op0=ALU.mult, op1=ALU.add)

        # ---- attention ----
        xn_sb = sb.tile([128, BLK, S], FP32R, name="xn_sb")
        layernorm(x_sb, xn_sb, "ln1")

        qk_sb = sb.tile([128, 4, S], FP32R)
        for mt in range(4):
            for (qo, qw) in QC:
                p = psmm.tile([128, 512], FP32, tag="mm")
                for blk in range(BLK):
                    nc.tensor.matmul(p[:, :qw],
                                     wqkT_sb[:, blk, mt * 128:(mt + 1) * 128],
                                     xn_sb[:, blk, qo:qo + qw],
                                     start=(blk == 0), stop=(blk == BLK - 1))
                nc.scalar.activation(out=qk_sb[:, mt, qo:qo + qw], in_=p[:, :qw],
                                     func=ACT.Identity, bias=bqk_sb[:, mt:mt + 1])

        v_sb = sb.tile([128, NKT, HEADS, 65], FP32R)
        nc.vector.tensor_copy(
            out=v_sb[:, :, :, 64:65],
            in_=ones_f[:, 0:1, None, None].to_broadcast([128, NKT, HEADS, 1]))
        for kt, (ko, kw) in enumerate(KT):
            p = psmm.tile([128, 512], FP32, tag="mm")
            for blk in range(BLK):
                nc.tensor.matmul(p[:kw, :C], xn_sb[:, blk, ko:ko + kw],
                                 wvT_sb[:, blk, :],
                                 start=(blk == 0), stop=(blk == BLK - 1))
            nc.vector.tensor_tensor(
                v_sb[:kw, kt, :, 0:64],
                p[:kw, :C].rearrange("p (h d) -> p h d", h=HEADS),
                bv_sb[:kw].rearrange("p (h d) -> p h d", h=HEADS),
                ALU.add)

        attn_sb = sb.tile([128, BLK, S], FP32R, tag="attn")
        for (qo, qw) in QC:
            for h in range(HEADS):
                pr = slice((h % 2) * 64, (h % 2) * 64 + 64)
                qblk = h // 2
                exp_c = expp.tile([128, NKT, 512], FP32R, tag="exp")
                for kt, (ko, kw) in enumerate(KT):
                    ps_sc = pssc.tile([128, 512], FP32, tag="sc")
                    nc.tensor.matmul(ps_sc[:kw, :qw],
                                     qk_sb[pr, 2 + qblk, ko:ko + kw],
                                     qk_sb[pr, qblk, qo:qo + qw],
                                     start=True, stop=True)
                    nc.scalar.activation(out=exp_c[:kw, kt, :qw], in_=ps_sc[:kw, :qw],
                                         func=ACT.Exp,
                                         bias=kb_sb[:kw, kt:kt + 1], scale=0.125)
                ps_o = pso.tile([65, 512], FP32, tag="o")
                for kt, (ko, kw) in enumerate(KT):
                    nc.tensor.matmul(ps_o[:, :qw], v_sb[:kw, kt, h, :],
                                     exp_c[:kw, kt, :qw],
                                     start=(kt == 0), stop=(kt == NKT - 1))
                rec_f = scr.tile([1, 512], FP32, tag="s512")
                nc.vector.reciprocal(out=rec_f[:, :qw], in_=ps_o[64:65, :qw])
                recd = scr1.tile([1, 512], FP32R, tag="recd")
                nc.vector.tensor_copy(out=recd[:, :qw], in_=rec_f[:, :qw])
                rep_d = psrep.tile([128, 512], FP32, tag="rep")
                nc.tensor.matmul(rep_d[0:64, :qw], ones_m[:, 0:64], recd[:, :qw],
                                 start=True, stop=True)
                onum = scr1.tile([64, 512], FP32, tag="onum")
                nc.scalar.activation(out=onum[:, :qw], in_=ps_o[0:64, :qw],
                                     func=ACT.Copy)
                nc.vector.tensor_tensor(attn_sb[pr, qblk, qo:qo + qw],
                                        onum[:, :qw], rep_d[0:64, :qw], ALU.mult)

        x2_sb = sb.tile([128, BLK, S], FP32)
        for mt in range(BLK):
            for (qo, qw) in QC:
                p = psmm.tile([128, 512], FP32, tag="mm")
                for blk in range(BLK):
                    nc.tensor.matmul(p[:, :qw],
                                     woT_sb[:, blk, mt * 128:(mt + 1) * 128],
                                     attn_sb[:, blk, qo:qo + qw],
                                     start=(blk == 0), stop=(blk == BLK - 1))
                t = scr.tile([128, 512], FP32, tag="t512")
                nc.scalar.activation(out=t[:, :qw], in_=p[:, :qw],
                                     func=ACT.Identity, bias=bo_sb[:, mt:mt + 1])
                nc.vector.tensor_tensor(x2_sb[:, mt, qo:qo + qw], t[:, :qw],
                                        x_sb[:, mt, qo:qo + qw], ALU.add)

        # ---- FFN ----
        xn2_sb = sb.tile([128, BLK, S], FP32R, tag="attn")  # reuse attn buffer
        layernorm(x2_sb, xn2_sb, "ln2")

        y_sb = sb.tile([128, BLK, S], FP32, tag="xy")       # reuse x buffer
        for (qo, qw) in QC:
            h1_full = expp.tile([128, NKT, 512], FP32R, tag="exp", name="h1_full")
            h1_c = h1_full[:, :FB, :]
            for mt in range(FB):
                p = psmm.tile([128, 512], FP32, tag="mm")
                for blk in range(BLK):
                    nc.tensor.matmul(p[:, :qw],
                                     w1T_sb[:, blk, mt * 128:(mt + 1) * 128],
                                     xn2_sb[:, blk, qo:qo + qw],
                                     start=(blk == 0), stop=(blk == BLK - 1))
                nc.scalar.activation(out=h1_c[:, mt, :qw], in_=p[:, :qw],
                                     func=ACT.Gelu, bias=b1_sb[:, mt:mt + 1])
            for mt in range(BLK):
                p = psmm.tile([128, 512], FP32, tag="mm")
                for kb in range(FB):
                    nc.tensor.matmul(p[:, :qw],
                                     w2T_sb[:, kb, mt * 128:(mt + 1) * 128],
                                     h1_c[:, kb, :qw],
                                     start=(kb == 0), stop=(kb == FB - 1))
                t = scr.tile([128, 512], FP32, tag="t512")
                nc.scalar.activation(out=t[:, :qw], in_=p[:, :qw],
                                     func=ACT.Identity, bias=b2_sb[:, mt:mt + 1])
                nc.vector.tensor_tensor(y_sb[:, mt, qo:qo + qw], t[:, :qw],
                                        x2_sb[:, mt, qo:qo + qw], ALU.add)

        nc.sync.dma_start(yg_d.rearrange("(blk p) t -> p blk t", p=128), y_sb)
    return nc


# ----------------------------------------------------------------------------
# Host orchestration
# ----------------------------------------------------------------------------
_CACHE = {}


def _get_agent_nc():
    if "agent" not in _CACHE:
        nc = build_agent_nc()
        nc.finalize()
        _CACHE["agent"] = nc
    return _CACHE["agent"]


def _get_mixer_nc(S):
    key = ("mixer", S)
    if key not in _CACHE:
        nc = build_mixer_nc(S)
        nc.finalize()
        _CACHE[key] = nc
    return _CACHE[key]


def kernel(f_ir, f_vis, aw1, ab1, aw2, ab2,
           ir_lng, ir_lnb, ir_wqkv, ir_bqkv, ir_wo, ir_bo, ir_w1, ir_b1,
           ir_w2, ir_b2,
           vis_lng, vis_lnb, vis_wqkv, vis_bqkv, vis_wo, vis_bo, vis_w1,
           vis_b1, vis_w2, vis_b2):
    f_ir = np.ascontiguousarray(f_ir, np.float32)
    f_vis = np.ascontiguousarray(f_vis, np.float32)

    # ---- launch A: routing logits + base canvas, token-parallel over 8 cores
    fir_n = f_ir.reshape(B, C, N)
    fvis_n = f_vis.reshape(B, C, N)
    X = np.concatenate([fir_n, fvis_n], axis=1)            # [B, 512, N]
    Xf = X.reshape(B, CIN, 2, TPC).transpose(0, 2, 1, 3).reshape(8, CIN, TPC)
    aw1t = np.ascontiguousarray(aw1.T, np.float32)
    aw2t = np.ascontiguousarray(aw2[0], np.float32)
    ab1 = np.ascontiguousarray(ab1, np.float32)
    ab2 = np.ascontiguousarray(ab2, np.float32)

    nc_a = _get_agent_nc()
    in_maps = [dict(x=np.ascontiguousarray(Xf[i]), aw1t=aw1t, ab1=ab1,
                    aw2t=aw2t, ab2=ab2) for i in range(8)]
    ra = run_bass_kernel_spmd(nc_a, in_maps, list(range(8)))
    logits = np.stack([ra.results[i]["logits"] for i in range(8)])
    logits = logits.reshape(B, 2 * TPC)                    # [B, N]
    base = np.stack([ra.results[i]["base"] for i in range(8)])
    base = base.reshape(B, 2, C, TPC).transpose(0, 2, 1, 3).reshape(B, C, N)

    # ---- host routing decision (reference semantics)
    mask = (logits > 0)
    counts = mask.sum(1)
    sel = np.empty_like(mask)
    for b in range(B):
        if counts[b] < MIN_TOK:
            top = np.argsort(-logits[b], kind="stable")[:MIN_TOK]
            s = np.zeros(N, bool)
            s[top] = True
            sel[b] = s
        else:
            sel[b] = mask[b]
    idxs = [np.where(sel[b])[0] for b in range(B)]
    s_max = max(len(i) for i in idxs)
    S = S_PAD_DEFAULT if s_max <= S_PAD_DEFAULT else ((s_max + 63) // 64) * 64
    NKT = len(_chunks(S, 128))

    # ---- launch B: one (batch, modality) mixer per core
    nc_b = _get_mixer_nc(S)
    in_maps_b = []
    metas = []
    for b in range(B):
        idx = idxs[b]
        Sb = len(idx)
        kbias = np.full((128 * NKT,), np.float32(-1e9), np.float32)
        kbias[:Sb] = 0.0
        for mod, fm, pfx in (("ir", fir_n[b], "ir"), ("vis", fvis_n[b], "vis")):
            params = {
                "lng": ir_lng if pfx == "ir" else vis_lng,
                "lnb": ir_lnb if pfx == "ir" else vis_lnb,
                "wqkv": ir_wqkv if pfx == "ir" else vis_wqkv,
                "bqkv": ir_bqkv if pfx == "ir" else vis_bqkv,
                "wo": ir_wo if pfx == "ir" else vis_wo,
                "bo": ir_bo if pfx == "ir" else vis_bo,
                "w1": ir_w1 if pfx == "ir" else vis_w1,
                "b1": ir_b1 if pfx == "ir" else vis_b1,
                "w2": ir_w2 if pfx == "ir" else vis_w2,
                "b2": ir_b2 if pfx == "ir" else vis_b2,
            }
            xg = np.zeros((C, S), np.float32)
            xg[:, :Sb] = fm[:, idx]
            wqkv = np.asarray(params["wqkv"], np.float32)
            im = dict(
                xg=xg, kbias=kbias,
                lng=np.ascontiguousarray(params["lng"], np.float32),
                lnb=np.ascontiguousarray(params["lnb"], np.float32),
                wqkT=np.ascontiguousarray(wqkv[:512].T),
                bqk=np.ascontiguousarray(params["bqkv"][:512], np.float32),
                wvT=np.ascontiguousarray(wqkv[512:].T),
                bv=np.ascontiguousarray(params["bqkv"][512:], np.float32),
                woT=np.ascontiguousarray(np.asarray(params["wo"], np.float32).T),
                bo=np.ascontiguousarray(params["bo"], np.float32),
                w1T=np.ascontiguousarray(np.asarray(params["w1"], np.float32).T),
                b1=np.ascontiguousarray(params["b1"], np.float32),
                w2T=np.ascontiguousarray(np.asarray(params["w2"], np.float32).T),
                b2=np.ascontiguousarray(params["b2"], np.float32),
            )
            in_maps_b.append(im)
            metas.append((b, mod, idx))
    rb = run_bass_kernel_spmd(nc_b, in_maps_b, list(range(8)))

    # ---- host scatter-combine
    out = base  # [B, C, N]; refined overwrites selected positions
    for ci in range(0, 8, 2):
        b, _, idx = metas[ci]
        Sb = len(idx)
        ri = rb.results[ci]["yg"][:, :Sb]
        rv = rb.results[ci + 1]["yg"][:, :Sb]
        refined = (ri + rv) * mask[b, idx].astype(np.float32)[None, :]
        out[b][:, idx] = refined
    return out.reshape(B, C, H, W)


# revision 6
# speedup vs baseline: 1.0901x; 1.0625x over previous
"""Trainium2 Bass kernel for nn_DynamicFusionModule (moe_routing).

Structure (8 NeuronCores, SPMD):
  Launch A (routing): the 9216 pixels (B=4 x N=2304) are split 8 ways; each
    core runs the SamplingAgent MLP (512->512 silu ->1) on its 1152 pixels in
    fp32r and also emits base = f_ir + f_vis for its slice.
  Host: mask = logits > 0 per batch; top-64 fallback exactly as the reference;
    selected indices are gathered per batch.
  Launch B (experts): one core per (batch, modality) runs the full MixerBlock
    (pre-LN MHA over the selected tokens + pre-LN FFN) on the gathered,
    padded-to-S_PAD token set, channel-major, fp32r matmuls.
  Host: scatter refined = (ri + rv) * mask into the base canvas.

All shapes are hardcoded for the fixed problem instance:
  f_ir, f_vis: [4, 256, 48, 48] fp32.
"""
import math
from contextlib import ExitStack

import numpy as np

import concourse.bass as bass
from concourse import bacc
import concourse.mybir as mybir
import concourse.tile as tile
from concourse.bass_utils import run_bass_kernel_spmd

FP32 = mybir.dt.float32
FP32R = mybir.dt.float32r
ACT = mybir.ActivationFunctionType
ALU = mybir.AluOpType

B = 4
C = 256        # model dim
H = W = 48
N = H * W      # 2304 tokens per batch
HEADS = 4
F = 1024       # FFN hidden
BLK = C // 128  # 2
FB = F // 128   # 8
CIN = 512      # agent input channels
HID = 512      # agent hidden
TPC = (B * N) // 8  # agent tokens per core = 1152
MIN_TOK = 64
S_PAD_DEFAULT = 1472  # >= max selected count (1451 for the fixed seed), mult of 64


def _chunks(total, size):
    out, o = [], 0
    while o < total:
        w = min(size, total - o)
        out.append((o, w))
        o += w
    return out


# ----------------------------------------------------------------------------
# Launch A: agent logits + base canvas
# ----------------------------------------------------------------------------
def build_agent_nc():
    nc = bacc.Bacc("TRN2", target_bir_lowering=False)
    x_d = nc.declare_dram_parameter("x", [CIN, TPC], FP32, isOutput=False)
    aw1t_d = nc.declare_dram_parameter("aw1t", [CIN, HID], FP32, isOutput=False)
    ab1_d = nc.declare_dram_parameter("ab1", [HID], FP32, isOutput=False)
    aw2t_d = nc.declare_dram_parameter("aw2t", [HID], FP32, isOutput=False)
    ab2_d = nc.declare_dram_parameter("ab2", [1], FP32, isOutput=False)
    lg_out = nc.declare_dram_parameter("logits", [TPC], FP32, isOutput=True)
    base_out = nc.declare_dram_parameter("base", [C, TPC], FP32, isOutput=True)

    KO = CIN // 128  # 4
    MO = HID // 128  # 4
    TCH = 384
    NT = TPC // TCH  # 3

    with tile.TileContext(nc) as tc, ExitStack() as ctx:
        sb = ctx.enter_context(tc.tile_pool(name="sb", bufs=1))
        xin = ctx.enter_context(tc.tile_pool(name="xin", bufs=2))
        ps = ctx.enter_context(tc.tile_pool(name="ps", bufs=2, space="PSUM"))

        w1_sb = sb.tile([128, KO, HID], FP32R)
        nc.gpsimd.dma_start(out=w1_sb,
                            in_=aw1t_d.rearrange("(ko p) m -> p ko m", p=128))
        b1_sb = sb.tile([128, MO], FP32)
        nc.sync.dma_start(b1_sb, ab1_d.rearrange("(mo p) -> p mo", p=128))
        w2_sb = sb.tile([128, MO], FP32R)
        nc.gpsimd.dma_start(out=w2_sb, in_=aw2t_d.rearrange("(mo p) -> p mo", p=128))
        ab2_sb = sb.tile([1, 1], FP32)
        nc.sync.dma_start(ab2_sb, ab2_d.rearrange("(a o) -> a o", a=1))

        x_r = x_d.rearrange("(ko p) t -> p ko t", p=128)
        base_r = base_out.rearrange("(blk p) t -> p blk t", p=128)
        lg_sb = sb.tile([1, TPC], FP32)
        # token-chunk pipeline: DMA(t+1) overlaps compute(t)
        for t in range(NT):
            tsl = slice(t * TCH, (t + 1) * TCH)
            x_f = xin.tile([128, KO, TCH], FP32, tag="x_f")
            nc.sync.dma_start(x_f, x_r[:, :, tsl])
            x_sb = xin.tile([128, KO, TCH], FP32R, tag="x_r")
            nc.vector.tensor_copy(out=x_sb, in_=x_f)
            # base = f_ir + f_vis for this slice (blocks 0,1 + blocks 2,3)
            base_sb = xin.tile([128, 2, TCH], FP32, tag="base")
            nc.vector.tensor_tensor(base_sb, x_f[:, 0:2], x_f[:, 2:4], ALU.add)
            nc.sync.dma_start(base_r[:, :, tsl], base_sb)

            h1_sb = xin.tile([128, MO, TCH], FP32R, tag="h1")
            for mo in range(MO):
                p = ps.tile([128, TCH], FP32, tag="acc")
                for ko in range(KO):
                    nc.tensor.matmul(
                        p, w1_sb[:, ko, mo * 128:(mo + 1) * 128],
                        x_sb[:, ko, :],
                        start=(ko == 0), stop=(ko == KO - 1))
                nc.scalar.activation(
                    out=h1_sb[:, mo, :], in_=p,
                    func=ACT.Silu, bias=b1_sb[:, mo:mo + 1], scale=1.0)

            p2 = ps.tile([1, TCH], FP32, tag="acc2")
            for mo in range(MO):
                nc.tensor.matmul(
                    p2, w2_sb[:, mo:mo + 1], h1_sb[:, mo, :],
                    start=(mo == 0), stop=(mo == MO - 1))
            nc.scalar.activation(out=lg_sb[:, tsl], in_=p2,
                                 func=ACT.Identity, bias=ab2_sb)
        nc.sync.dma_start(lg_out.rearrange("(o t) -> o t", o=1), lg_sb)
    return nc


# ----------------------------------------------------------------------------
# Launch B: MixerBlock on S gathered tokens (see dev notes in docstring)
# ----------------------------------------------------------------------------
def build_mixer_nc(S: int):
    KT = _chunks(S, 128)
    QC = _chunks(S, 512)
    NKT = len(KT)
    KBIAS_LEN = 128 * NKT

    nc = bacc.Bacc("TRN2", target_bir_lowering=False)
    xg_d = nc.declare_dram_parameter("xg", [C, S], FP32, isOutput=False)
    kb_d = nc.declare_dram_parameter("kbias", [KBIAS_LEN], FP32, isOutput=False)
    lng_d = nc.declare_dram_parameter("lng", [C], FP32, isOutput=False)
    lnb_d = nc.declare_dram_parameter("lnb", [C], FP32, isOutput=False)
    wqkT_d = nc.declare_dram_parameter("wqkT", [C, 512], FP32, isOutput=False)
    bqk_d = nc.declare_dram_parameter("bqk", [512], FP32, isOutput=False)
    wvT_d = nc.declare_dram_parameter("wvT", [C, C], FP32, isOutput=False)
    bv_d = nc.declare_dram_parameter("bv", [C], FP32, isOutput=False)
    woT_d = nc.declare_dram_parameter("woT", [C, C], FP32, isOutput=False)
    bo_d = nc.declare_dram_parameter("bo", [C], FP32, isOutput=False)
    w1T_d = nc.declare_dram_parameter("w1T", [C, F], FP32, isOutput=False)
    b1_d = nc.declare_dram_parameter("b1", [F], FP32, isOutput=False)
    w2T_d = nc.declare_dram_parameter("w2T", [F, C], FP32, isOutput=False)
    b2_d = nc.declare_dram_parameter("b2", [C], FP32, isOutput=False)
    yg_d = nc.declare_dram_parameter("yg", [C, S], FP32, isOutput=True)

    with tile.TileContext(nc) as tc, ExitStack() as ctx:
        sb = ctx.enter_context(tc.tile_pool(name="sb", bufs=1))
        scr = ctx.enter_context(tc.tile_pool(name="scr", bufs=2))
        scr1 = ctx.enter_context(tc.tile_pool(name="scr1", bufs=1))
        expp = ctx.enter_context(tc.tile_pool(name="expp", bufs=2))
        psmm = ctx.enter_context(tc.tile_pool(name="psmm", bufs=4, space="PSUM"))
        pso = ctx.enter_context(tc.tile_pool(name="pso", bufs=2, space="PSUM"))
        psrep = ctx.enter_context(tc.tile_pool(name="psrep", bufs=2, space="PSUM"))

        x_sb = sb.tile([128, BLK, S], FP32, tag="xy")
        xg_r = xg_d.rearrange("(blk p) t -> p blk t", p=128)
        for (qo, qw) in QC:
            nc.sync.dma_start(x_sb[:, :, qo:qo + qw], xg_r[:, :, qo:qo + qw])
        kb_sb = sb.tile([128, NKT], FP32)
        nc.sync.dma_start(kb_sb, kb_d.rearrange("(kt p) -> p kt", p=128))
        lng_sb = sb.tile([128, BLK], FP32)
        nc.sync.dma_start(lng_sb, lng_d.rearrange("(blk p) -> p blk", p=128))
        lnb_sb = sb.tile([128, BLK], FP32)
        nc.sync.dma_start(lnb_sb, lnb_d.rearrange("(blk p) -> p blk", p=128))
        bqk_sb = sb.tile([128, 4], FP32)
        nc.sync.dma_start(bqk_sb, bqk_d.rearrange("(m p) -> p m", p=128))
        bo_sb = sb.tile([128, BLK], FP32)
        nc.sync.dma_start(bo_sb, bo_d.rearrange("(m p) -> p m", p=128))
        b1_sb = sb.tile([128, FB], FP32)
        nc.sync.dma_start(b1_sb, b1_d.rearrange("(m p) -> p m", p=128))
        b2_sb = sb.tile([128, BLK], FP32)
        nc.sync.dma_start(b2_sb, b2_d.rearrange("(m p) -> p m", p=128))
        bv_ap = bv_d[:]
        bv_bc_src = bass.AP(tensor=bv_ap.tensor, offset=bv_ap.offset,
                            ap=[[0, 128]] + [list(p) for p in bv_ap.ap])
        bv_sb = sb.tile([128, C], FP32)
        nc.gpsimd.dma_start(out=bv_sb, in_=bv_bc_src)

        def load_w(dram, cols, kblocks, nm):
            t_r = sb.tile([128, kblocks, cols], FP32R, name=nm)
            nc.gpsimd.dma_start(out=t_r,
                                in_=dram.rearrange("(kb p) m -> p kb m", p=128))
            return t_r

        wqkT_sb = load_w(wqkT_d, 512, BLK, "wqkT_sb")
        wvT_sb = load_w(wvT_d, C, BLK, "wvT_sb")
        woT_sb = load_w(woT_d, C, BLK, "woT_sb")
        w1T_sb = load_w(w1T_d, F, BLK, "w1T_sb")
        w2T_sb = load_w(w2T_d, C, FB, "w2T_sb")

        ones_f = sb.tile([128, 128], FP32)
        nc.vector.memset(ones_f, 1.0)
        ones_m = sb.tile([1, 128], FP32R)   # lhsT for partition-replicate
        nc.vector.tensor_copy(out=ones_m, in_=ones_f[0:1])
        ones_k = sb.tile([128, 1], FP32R)   # lhsT for channel-sum
        nc.vector.tensor_copy(out=ones_k, in_=ones_f[:, 0:1])
        eps_sb = sb.tile([1, 1], FP32)
        nc.vector.memset(eps_sb, 1e-5)

        def layernorm(x_in, xn_out, uid):
            mean_r = sb.tile([1, S], FP32R, name=f"mean_{uid}")
            rstd_r = sb.tile([1, S], FP32R, name=f"rstd_{uid}")
            for (qo, qw) in QC:
                xr_c = scr.tile([128, BLK, 512], FP32R, tag="xr_c")
                nc.vector.tensor_copy(out=xr_c[:, :, :qw], in_=x_in[:, :, qo:qo + qw])
                xsq_c = scr.tile([128, BLK, 512], FP32R, tag="xsq_c")
                nc.scalar.activation(out=xsq_c[:, :, :qw], in_=x_in[:, :, qo:qo + qw],
                                     func=ACT.Square)
                ps_s = psrep.tile([128, 512], FP32, tag="rep")
                ps_q = psrep.tile([128, 512], FP32, tag="rep")
                for blk in range(BLK):
                    nc.tensor.matmul(ps_s[0:1, :qw], ones_k, xr_c[:, blk, :qw],
                                     start=(blk == 0), stop=(blk == BLK - 1))
                for blk in range(BLK):
                    nc.tensor.matmul(ps_q[0:1, :qw], ones_k, xsq_c[:, blk, :qw],
                                     start=(blk == 0), stop=(blk == BLK - 1))
                nc.scalar.mul(out=mean_r[:, qo:qo + qw], in_=ps_s[0:1, :qw],
                              mul=1.0 / C)
                m2 = scr.tile([1, 512], FP32, tag="s512")
                nc.vector.tensor_tensor(m2[:, :qw], mean_r[:, qo:qo + qw],
                                        mean_r[:, qo:qo + qw], ALU.mult)
                var_c = scr.tile([1, 512], FP32, tag="s512")
                nc.scalar.mul(out=var_c[:, :qw], in_=ps_q[0:1, :qw], mul=1.0 / C)
                nc.vector.tensor_tensor(var_c[:, :qw], var_c[:, :qw], m2[:, :qw],
                                        ALU.subtract)
                sd_c = scr.tile([1, 512], FP32, tag="s512")
                nc.scalar.activation(out=sd_c[:, :qw], in_=var_c[:, :qw],
                                     func=ACT.Sqrt, bias=eps_sb)
                rec_f = scr.tile([1, 512], FP32, tag="s512")
                nc.vector.reciprocal(out=rec_f[:, :qw], in_=sd_c[:, :qw])
                nc.vector.tensor_copy(out=rstd_r[:, qo:qo + qw], in_=rec_f[:, :qw])
            for (qo, qw) in QC:
                rep_m = psrep.tile([128, 512], FP32, tag="rep")
                nc.tensor.matmul(rep_m[:, :qw], ones_m, mean_r[:, qo:qo + qw],
                                 start=True, stop=True)
                rep_s = psrep.tile([128, 512], FP32, tag="rep")
                nc.tensor.matmul(rep_s[:, :qw], ones_m, rstd_r[:, qo:qo + qw],
                                 start=True, stop=True)
                for blk in range(BLK):
                    t = scr.tile([128, 512], FP32, tag="t512")
                    nc.vector.tensor_tensor(t[:, :qw], x_in[:, blk, qo:qo + qw],
                                            rep_m[:, :qw], ALU.subtract)
                    nc.vector.tensor_tensor(t[:, :qw], t[:, :qw], rep_s[:, :qw],
                                            ALU.mult)
                    nc.vector.tensor_scalar(
                        out=xn_out[:, blk, qo:qo + qw], in0=t[:, :qw],
                        scalar1=lng_sb[:, blk:blk + 1], scalar2=lnb_sb[:, blk:blk + 1],
                        op0=ALU.mult, op1=ALU.add)

        # ---- attention ----
        xn_sb = sb.tile([128, BLK, S], FP32R, name="xn_sb")
        layernorm(x_sb, xn_sb, "ln1")

        qk_sb = sb.tile([128, 4, S], FP32R)
        for mt in range(4):
            for (qo, qw) in QC:
                p = psmm.tile([128, 512], FP32, tag="mm")
                for blk in range(BLK):
                    nc.tensor.matmul(p[:, :qw],
                                     wqkT_sb[:, blk, mt * 128:(mt + 1) * 128],
                                     xn_sb[:, blk, qo:qo + qw],
                                     start=(blk == 0), stop=(blk == BLK - 1))
                nc.scalar.activation(out=qk_sb[:, mt, qo:qo + qw], in_=p[:, :qw],
                                     func=ACT.Identity, bias=bqk_sb[:, mt:mt + 1])

        v_sb = sb.tile([128, NKT, HEADS, 65], FP32R)
        nc.vector.tensor_copy(
            out=v_sb[:, :, :, 64:65],
            in_=ones_f[:, 0:1, None, None].to_broadcast([128, NKT, HEADS, 1]))
        for kt, (ko, kw) in enumerate(KT):
            p = psmm.tile([128, 512], FP32, tag="mm")
            for blk in range(BLK):
                nc.tensor.matmul(p[:kw, :C], xn_sb[:, blk, ko:ko + kw],
                                 wvT_sb[:, blk, :],
                                 start=(blk == 0), stop=(blk == BLK - 1))
            nc.vector.tensor_tensor(
                v_sb[:kw, kt, :, 0:64],
                p[:kw, :C].rearrange("p (h d) -> p h d", h=HEADS),
                bv_sb[:kw].rearrange("p (h d) -> p h d", h=HEADS),
                ALU.add)

        attn_sb = sb.tile([128, BLK, S], FP32R, tag="attn")
        for (qo, qw) in QC:
            for h in range(HEADS):
                pr = slice((h % 2) * 64, (h % 2) * 64 + 64)
                qblk = h // 2
                exp_c = expp.tile([128, NKT, 512], FP32R, tag="exp")
                for kt, (ko, kw) in enumerate(KT):
                    ps_sc = psmm.tile([128, 512], FP32, tag="mm")
                    nc.tensor.matmul(ps_sc[:kw, :qw],
                                     qk_sb[pr, 2 + qblk, ko:ko + kw],
                                     qk_sb[pr, qblk, qo:qo + qw],
                                     start=True, stop=True)
                    nc.scalar.activation(out=exp_c[:kw, kt, :qw], in_=ps_sc[:kw, :qw],
                                         func=ACT.Exp,
                                         bias=kb_sb[:kw, kt:kt + 1], scale=0.125)
                ps_o = pso.tile([65, 512], FP32, tag="o")
                for kt, (ko, kw) in enumerate(KT):
                    nc.tensor.matmul(ps_o[:, :qw], v_sb[:kw, kt, h, :],
                                     exp_c[:kw, kt, :qw],
                                     start=(kt == 0), stop=(kt == NKT - 1))
                rec_f = scr.tile([1, 512], FP32, tag="s512")
                nc.vector.reciprocal(out=rec_f[:, :qw], in_=ps_o[64:65, :qw])
                recd = scr1.tile([1, 512], FP32R, tag="recd")
                nc.vector.tensor_copy(out=recd[:, :qw], in_=rec_f[:, :qw])
                rep_d = psrep.tile([128, 512], FP32, tag="rep")
                nc.tensor.matmul(rep_d[0:64, :qw], ones_m[:, 0:64], recd[:, :qw],
                                 start=True, stop=True)
                onum = scr1.tile([64, 512], FP32, tag="onum")
                nc.scalar.activation(out=onum[:, :qw], in_=ps_o[0:64, :qw],
                                     func=ACT.Copy)
                nc.vector.tensor_tensor(attn_sb[pr, qblk, qo:qo + qw],
                                        onum[:, :qw], rep_d[0:64, :qw], ALU.mult)

        x2_sb = sb.tile([128, BLK, S], FP32)
        for (qo, qw) in QC:
            for mt in range(BLK):
                p = psmm.tile([128, 512], FP32, tag="mm")
                for blk in range(BLK):
                    nc.tensor.matmul(p[:, :qw],
                                     woT_sb[:, blk, mt * 128:(mt + 1) * 128],
                                     attn_sb[:, blk, qo:qo + qw],
                                     start=(blk == 0), stop=(blk == BLK - 1))
                t = scr.tile([128, 512], FP32, tag="t512")
                nc.scalar.activation(out=t[:, :qw], in_=p[:, :qw],
                                     func=ACT.Identity, bias=bo_sb[:, mt:mt + 1])
                nc.vector.tensor_tensor(x2_sb[:, mt, qo:qo + qw], t[:, :qw],
                                        x_sb[:, mt, qo:qo + qw], ALU.add)

        # ---- FFN ----
        xn2_sb = sb.tile([128, BLK, S], FP32R, tag="attn")  # reuse attn buffer
        layernorm(x2_sb, xn2_sb, "ln2")

        y_sb = sb.tile([128, BLK, S], FP32, tag="xy")       # reuse x buffer
        yg_r = yg_d.rearrange("(blk p) t -> p blk t", p=128)
        for (qo, qw) in QC:
            h1_full = expp.tile([128, NKT, 512], FP32R, tag="exp", name="h1_full")
            h1_c = h1_full[:, :FB, :]
            for mt in range(FB):
                p = psmm.tile([128, 512], FP32, tag="mm")
                for blk in range(BLK):
                    nc.tensor.matmul(p[:, :qw],
                                     w1T_sb[:, blk, mt * 128:(mt + 1) * 128],
                                     xn2_sb[:, blk, qo:qo + qw],
                                     start=(blk == 0), stop=(blk == BLK - 1))
                nc.scalar.activation(out=h1_c[:, mt, :qw], in_=p[:, :qw],
                                     func=ACT.Gelu, bias=b1_sb[:, mt:mt + 1])
            for mt in range(BLK):
                p = psmm.tile([128, 512], FP32, tag="mm")
                for kb in range(FB):
                    nc.tensor.matmul(p[:, :qw],
                                     w2T_sb[:, kb, mt * 128:(mt + 1) * 128],
                                     h1_c[:, kb, :qw],
                                     start=(kb == 0), stop=(kb == FB - 1))
                t = scr.tile([128, 512], FP32, tag="t512")
                nc.scalar.activation(out=t[:, :qw], in_=p[:, :qw],
                                     func=ACT.Identity, bias=b2_sb[:, mt:mt + 1])
                nc.vector.tensor_tensor(y_sb[:, mt, qo:qo + qw], t[:, :qw],
                                        x2_sb[:, mt, qo:qo + qw], ALU.add)
            nc.sync.dma_start(yg_r[:, :, qo:qo + qw], y_sb[:, :, qo:qo + qw])
    return nc


# ----------------------------------------------------------------------------
# Host orchestration
# ----------------------------------------------------------------------------
_CACHE = {}


def _get_agent_nc():
    if "agent" not in _CACHE:
        nc = build_agent_nc()
        nc.finalize()
        _CACHE["agent"] = nc
    return _CACHE["agent"]


def _get_mixer_nc(S):
    key = ("mixer", S)
    if key not in _CACHE:
        nc = build_mixer_nc(S)
        nc.finalize()
        _CACHE[key] = nc
    return _CACHE[key]


def kernel(f_ir, f_vis, aw1, ab1, aw2, ab2,
           ir_lng, ir_lnb, ir_wqkv, ir_bqkv, ir_wo, ir_bo, ir_w1, ir_b1,
           ir_w2, ir_b2,
           vis_lng, vis_lnb, vis_wqkv, vis_bqkv, vis_wo, vis_bo, vis_w1,
           vis_b1, vis_w2, vis_b2):
    f_ir = np.ascontiguousarray(f_ir, np.float32)
    f_vis = np.ascontiguousarray(f_vis, np.float32)

    # ---- launch A: routing logits + base canvas, token-parallel over 8 cores
    fir_n = f_ir.reshape(B, C, N)
    fvis_n = f_vis.reshape(B, C, N)
    X = np.concatenate([fir_n, fvis_n], axis=1)            # [B, 512, N]
    Xf = X.reshape(B, CIN, 2, TPC).transpose(0, 2, 1, 3).reshape(8, CIN, TPC)
    aw1t = np.ascontiguousarray(aw1.T, np.float32)
    aw2t = np.ascontiguousarray(aw2[0], np.float32)
    ab1 = np.ascontiguousarray(ab1, np.float32)
    ab2 = np.ascontiguousarray(ab2, np.float32)

    nc_a = _get_agent_nc()
    in_maps = [dict(x=np.ascontiguousarray(Xf[i]), aw1t=aw1t, ab1=ab1,
                    aw2t=aw2t, ab2=ab2) for i in range(8)]
    ra = run_bass_kernel_spmd(nc_a, in_maps, list(range(8)))
    logits = np.stack([ra.results[i]["logits"] for i in range(8)])
    logits = logits.reshape(B, 2 * TPC)                    # [B, N]
    base = np.stack([ra.results[i]["base"] for i in range(8)])
    base = base.reshape(B, 2, C, TPC).transpose(0, 2, 1, 3).reshape(B, C, N)

    # ---- host routing decision (reference semantics)
    mask = (logits > 0)
    counts = mask.sum(1)
    sel = np.empty_like(mask)
    for b in range(B):
        if counts[b] < MIN_TOK:
            top = np.argsort(-logits[b], kind="stable")[:MIN_TOK]
            s = np.zeros(N, bool)
            s[top] = True
            sel[b] = s
        else:
            sel[b] = mask[b]
    idxs = [np.where(sel[b])[0] for b in range(B)]
    s_max = max(len(i) for i in idxs)
    S = S_PAD_DEFAULT if s_max <= S_PAD_DEFAULT else ((s_max + 63) // 64) * 64
    NKT = len(_chunks(S, 128))

    # ---- launch B: one (batch, modality) mixer per core
    nc_b = _get_mixer_nc(S)
    in_maps_b = []
    metas = []
    for b in range(B):
        idx = idxs[b]
        Sb = len(idx)
        kbias = np.full((128 * NKT,), np.float32(-1e9), np.float32)
        kbias[:Sb] = 0.0
        for mod, fm, pfx in (("ir", fir_n[b], "ir"), ("vis", fvis_n[b], "vis")):
            params = {
                "lng": ir_lng if pfx == "ir" else vis_lng,
                "lnb": ir_lnb if pfx == "ir" else vis_lnb,
                "wqkv": ir_wqkv if pfx == "ir" else vis_wqkv,
                "bqkv": ir_bqkv if pfx == "ir" else vis_bqkv,
                "wo": ir_wo if pfx == "ir" else vis_wo,
                "bo": ir_bo if pfx == "ir" else vis_bo,
                "w1": ir_w1 if pfx == "ir" else vis_w1,
                "b1": ir_b1 if pfx == "ir" else vis_b1,
                "w2": ir_w2 if pfx == "ir" else vis_w2,
                "b2": ir_b2 if pfx == "ir" else vis_b2,
            }
            xg = np.zeros((C, S), np.float32)
            xg[:, :Sb] = fm[:, idx]
            wqkv = np.asarray(params["wqkv"], np.float32)
            im = dict(
                xg=xg, kbias=kbias,
                lng=np.ascontiguousarray(params["lng"], np.float32),
                lnb=np.ascontiguousarray(params["lnb"], np.float32),
                wqkT=np.ascontiguousarray(wqkv[:512].T),
                bqk=np.ascontiguousarray(params["bqkv"][:512], np.float32),
                wvT=np.ascontiguousarray(wqkv[512:].T),
                bv=np.ascontiguousarray(params["bqkv"][512:], np.float32),
                woT=np.ascontiguousarray(np.asarray(params["wo"], np.float32).T),
                bo=np.ascontiguousarray(params["bo"], np.float32),
                w1T=np.ascontiguousarray(np.asarray(params["w1"], np.float32).T),
                b1=np.ascontiguousarray(params["b1"], np.float32),
                w2T=np.ascontiguousarray(np.asarray(params["w2"], np.float32).T),
                b2=np.ascontiguousarray(params["b2"], np.float32),
            )
            in_maps_b.append(im)
            metas.append((b, mod, idx))
    rb = run_bass_kernel_spmd(nc_b, in_maps_b, list(range(8)))

    # ---- host scatter-combine
    out = base  # [B, C, N]; refined overwrites selected positions
    for ci in range(0, 8, 2):
        b, _, idx = metas[ci]
        Sb = len(idx)
        ri = rb.results[ci]["yg"][:, :Sb]
        rv = rb.results[ci + 1]["yg"][:, :Sb]
        refined = (ri + rv) * mask[b, idx].astype(np.float32)[None, :]
        out[b][:, idx] = refined
    return out.reshape(B, C, H, W)


# revision 9
# speedup vs baseline: 1.1191x; 1.0266x over previous
"""Trainium2 Bass kernel for nn_DynamicFusionModule (moe_routing).

Structure (8 NeuronCores, SPMD):
  Launch A (routing): the 9216 pixels (B=4 x N=2304) are split 8 ways; each
    core runs the SamplingAgent MLP (512->512 silu ->1) on its 1152 pixels in
    fp32r and also emits base = f_ir + f_vis for its slice.
  Host: mask = logits > 0 per batch; top-64 fallback exactly as the reference;
    selected indices are gathered per batch.
  Launch B (experts): one core per (batch, modality) runs the full MixerBlock
    (pre-LN MHA over the selected tokens + pre-LN FFN) on the gathered,
    padded-to-S_PAD token set, channel-major, fp32r matmuls.
  Host: scatter refined = (ri + rv) * mask into the base canvas.

All shapes are hardcoded for the fixed problem instance:
  f_ir, f_vis: [4, 256, 48, 48] fp32.
"""
import math
from contextlib import ExitStack

import numpy as np

import concourse.bass as bass
from concourse import bacc
import concourse.mybir as mybir
import concourse.tile as tile
from concourse.bass_utils import run_bass_kernel_spmd

FP32 = mybir.dt.float32
FP32R = mybir.dt.float32r
BF16 = mybir.dt.bfloat16
ACT = mybir.ActivationFunctionType
ALU = mybir.AluOpType

B = 4
C = 256        # model dim
H = W = 48
N = H * W      # 2304 tokens per batch
HEADS = 4
F = 1024       # FFN hidden
BLK = C // 128  # 2
FB = F // 128   # 8
CIN = 512      # agent input channels
HID = 512      # agent hidden
TPC = (B * N) // 8  # agent tokens per core = 1152
MIN_TOK = 64
S_PAD_DEFAULT = 1472  # >= max selected count (1451 for the fixed seed), mult of 64


def _chunks(total, size):
    out, o = [], 0
    while o < total:
        w = min(size, total - o)
        out.append((o, w))
        o += w
    return out


# ----------------------------------------------------------------------------
# Launch A: agent logits + base canvas
# ----------------------------------------------------------------------------
def build_agent_nc():
    nc = bacc.Bacc("TRN2", target_bir_lowering=False)
    x_d = nc.declare_dram_parameter("x", [CIN, TPC], FP32, isOutput=False)
    aw1t_d = nc.declare_dram_parameter("aw1t", [CIN, HID], FP32, isOutput=False)
    ab1_d = nc.declare_dram_parameter("ab1", [HID], FP32, isOutput=False)
    aw2t_d = nc.declare_dram_parameter("aw2t", [HID], FP32, isOutput=False)
    ab2_d = nc.declare_dram_parameter("ab2", [1], FP32, isOutput=False)
    lg_out = nc.declare_dram_parameter("logits", [TPC], FP32, isOutput=True)
    base_out = nc.declare_dram_parameter("base", [C, TPC], FP32, isOutput=True)

    KO = CIN // 128  # 4
    MO = HID // 128  # 4
    TCH = 384
    NT = TPC // TCH  # 3

    with tile.TileContext(nc) as tc, ExitStack() as ctx:
        sb = ctx.enter_context(tc.tile_pool(name="sb", bufs=1))
        xin = ctx.enter_context(tc.tile_pool(name="xin", bufs=2))
        ps = ctx.enter_context(tc.tile_pool(name="ps", bufs=2, space="PSUM"))

        w1_sb = sb.tile([128, KO, HID], FP32R)
        nc.gpsimd.dma_start(out=w1_sb,
                            in_=aw1t_d.rearrange("(ko p) m -> p ko m", p=128))
        b1_sb = sb.tile([128, MO], FP32)
        nc.sync.dma_start(b1_sb, ab1_d.rearrange("(mo p) -> p mo", p=128))
        w2_sb = sb.tile([128, MO], FP32R)
        nc.gpsimd.dma_start(out=w2_sb, in_=aw2t_d.rearrange("(mo p) -> p mo", p=128))
        ab2_sb = sb.tile([1, 1], FP32)
        nc.sync.dma_start(ab2_sb, ab2_d.rearrange("(a o) -> a o", a=1))

        x_r = x_d.rearrange("(ko p) t -> p ko t", p=128)
        base_r = base_out.rearrange("(blk p) t -> p blk t", p=128)
        lg_sb = sb.tile([1, TPC], FP32)
        # token-chunk pipeline: DMA(t+1) overlaps compute(t)
        for t in range(NT):
            tsl = slice(t * TCH, (t + 1) * TCH)
            x_f = xin.tile([128, KO, TCH], FP32, tag="x_f")
            nc.sync.dma_start(x_f, x_r[:, :, tsl])
            x_sb = xin.tile([128, KO, TCH], FP32R, tag="x_r")
            nc.vector.tensor_copy(out=x_sb, in_=x_f)
            # base = f_ir + f_vis for this slice (blocks 0,1 + blocks 2,3)
            base_sb = xin.tile([128, 2, TCH], FP32, tag="base")
            nc.vector.tensor_tensor(base_sb, x_f[:, 0:2], x_f[:, 2:4], ALU.add)
            nc.sync.dma_start(base_r[:, :, tsl], base_sb)

            h1_sb = xin.tile([128, MO, TCH], FP32R, tag="h1")
            for mo in range(MO):
                p = ps.tile([128, TCH], FP32, tag="acc")
                for ko in range(KO):
                    nc.tensor.matmul(
                        p, w1_sb[:, ko, mo * 128:(mo + 1) * 128],
                        x_sb[:, ko, :],
                        start=(ko == 0), stop=(ko == KO - 1))
                nc.scalar.activation(
                    out=h1_sb[:, mo, :], in_=p,
                    func=ACT.Silu, bias=b1_sb[:, mo:mo + 1], scale=1.0)

            p2 = ps.tile([1, TCH], FP32, tag="acc2")
            for mo in range(MO):
                nc.tensor.matmul(
                    p2, w2_sb[:, mo:mo + 1], h1_sb[:, mo, :],
                    start=(mo == 0), stop=(mo == MO - 1))
            nc.scalar.activation(out=lg_sb[:, tsl], in_=p2,
                                 func=ACT.Identity, bias=ab2_sb)
        nc.sync.dma_start(lg_out.rearrange("(o t) -> o t", o=1), lg_sb)
    return nc


# ----------------------------------------------------------------------------
# Launch B: MixerBlock on S gathered tokens (see dev notes in docstring)
# ----------------------------------------------------------------------------
def build_mixer_nc(S: int):
    KT = _chunks(S, 128)
    QC = _chunks(S, 512)
    NKT = len(KT)
    KBIAS_LEN = 128 * NKT

    nc = bacc.Bacc("TRN2", target_bir_lowering=False)
    xg_d = nc.declare_dram_parameter("xg", [C, S], FP32, isOutput=False)
    kb_d = nc.declare_dram_parameter("kbias", [KBIAS_LEN], FP32, isOutput=False)
    lng_d = nc.declare_dram_parameter("lng", [C], FP32, isOutput=False)
    lnb_d = nc.declare_dram_parameter("lnb", [C], FP32, isOutput=False)
    wqkT_d = nc.declare_dram_parameter("wqkT", [C, 512], FP32, isOutput=False)
    bqk_d = nc.declare_dram_parameter("bqk", [512], FP32, isOutput=False)
    wvT_d = nc.declare_dram_parameter("wvT", [C, C], FP32, isOutput=False)
    bv_d = nc.declare_dram_parameter("bv", [C], FP32, isOutput=False)
    woT_d = nc.declare_dram_parameter("woT", [C, C], FP32, isOutput=False)
    bo_d = nc.declare_dram_parameter("bo", [C], FP32, isOutput=False)
    w1T_d = nc.declare_dram_parameter("w1T", [C, F], FP32, isOutput=False)
    b1_d = nc.declare_dram_parameter("b1", [F], FP32, isOutput=False)
    w2T_d = nc.declare_dram_parameter("w2T", [F, C], FP32, isOutput=False)
    b2_d = nc.declare_dram_parameter("b2", [C], FP32, isOutput=False)
    yg_d = nc.declare_dram_parameter("yg", [C, S], FP32, isOutput=True)

    with tile.TileContext(nc) as tc, ExitStack() as ctx:
        sb = ctx.enter_context(tc.tile_pool(name="sb", bufs=1))
        scr = ctx.enter_context(tc.tile_pool(name="scr", bufs=2))
        scr1 = ctx.enter_context(tc.tile_pool(name="scr1", bufs=1))
        expp = ctx.enter_context(tc.tile_pool(name="expp", bufs=2))
        psmm = ctx.enter_context(tc.tile_pool(name="psmm", bufs=4, space="PSUM"))
        pso = ctx.enter_context(tc.tile_pool(name="pso", bufs=1, space="PSUM"))
        psrep = ctx.enter_context(tc.tile_pool(name="psrep", bufs=1, space="PSUM"))

        x_sb = sb.tile([128, BLK, S], FP32, tag="xy")
        xg_r = xg_d.rearrange("(blk p) t -> p blk t", p=128)
        for (qo, qw) in QC:
            nc.sync.dma_start(x_sb[:, :, qo:qo + qw], xg_r[:, :, qo:qo + qw])
        kb_sb = sb.tile([128, NKT], FP32)
        nc.sync.dma_start(kb_sb, kb_d.rearrange("(kt p) -> p kt", p=128))
        lng_sb = sb.tile([128, BLK], FP32)
        nc.sync.dma_start(lng_sb, lng_d.rearrange("(blk p) -> p blk", p=128))
        lnb_sb = sb.tile([128, BLK], FP32)
        nc.sync.dma_start(lnb_sb, lnb_d.rearrange("(blk p) -> p blk", p=128))
        bqk_sb = sb.tile([128, 4], FP32)
        nc.sync.dma_start(bqk_sb, bqk_d.rearrange("(m p) -> p m", p=128))
        bo_sb = sb.tile([128, BLK], FP32)
        nc.sync.dma_start(bo_sb, bo_d.rearrange("(m p) -> p m", p=128))
        b1_sb = sb.tile([128, FB], FP32)
        nc.sync.dma_start(b1_sb, b1_d.rearrange("(m p) -> p m", p=128))
        b2_sb = sb.tile([128, BLK], FP32)
        nc.sync.dma_start(b2_sb, b2_d.rearrange("(m p) -> p m", p=128))
        bv_ap = bv_d[:]
        bv_bc_src = bass.AP(tensor=bv_ap.tensor, offset=bv_ap.offset,
                            ap=[[0, 128]] + [list(p) for p in bv_ap.ap])
        bv_sb = sb.tile([128, C], FP32)
        nc.gpsimd.dma_start(out=bv_sb, in_=bv_bc_src)

        def load_w(dram, cols, kblocks, nm):
            t_r = sb.tile([128, kblocks, cols], FP32R, name=nm)
            nc.gpsimd.dma_start(out=t_r,
                                in_=dram.rearrange("(kb p) m -> p kb m", p=128))
            return t_r

        wqkT_sb = load_w(wqkT_d, 512, BLK, "wqkT_sb")
        wvT_sb = load_w(wvT_d, C, BLK, "wvT_sb")
        woT_sb = load_w(woT_d, C, BLK, "woT_sb")
        w1T_sb = load_w(w1T_d, F, BLK, "w1T_sb")
        w2T_sb = sb.tile([128, FB, C], BF16, name="w2T_sb")
        nc.gpsimd.dma_start(out=w2T_sb, in_=w2T_d.rearrange("(kb p) m -> p kb m", p=128))

        ones_f = sb.tile([128, 128], FP32)
        nc.vector.memset(ones_f, 1.0)
        ones_m = sb.tile([1, 128], FP32R)   # lhsT for partition-replicate
        nc.vector.tensor_copy(out=ones_m, in_=ones_f[0:1])
        ones_kr = sb.tile([128, 128], FP32R)  # lhsT for replicated channel-sum
        nc.vector.tensor_copy(out=ones_kr, in_=ones_f)
        eps_sb = sb.tile([128, 1], FP32)
        nc.vector.memset(eps_sb, 1e-5)

        def layernorm(x_in, xn_out, uid):
            for (qo, qw) in QC:
                xr_c = scr.tile([128, BLK, 512], FP32R, tag="xr_c")
                nc.vector.tensor_copy(out=xr_c[:, :, :qw], in_=x_in[:, :, qo:qo + qw])
                xsq_c = scr.tile([128, BLK, 512], FP32R, tag="xsq_c")
                nc.scalar.activation(out=xsq_c[:, :, :qw], in_=x_in[:, :, qo:qo + qw],
                                     func=ACT.Square)
                # replicated sums: every output partition carries the full
                # channel-sum, so stats ops run 128 lanes wide and no separate
                # replicate matmul is needed
                ps_s = psmm.tile([128, 512], FP32, tag="mm")
                ps_q = psmm.tile([128, 512], FP32, tag="mm")
                for blk in range(BLK):
                    nc.tensor.matmul(ps_s[:, :qw], ones_kr, xr_c[:, blk, :qw],
                                     start=(blk == 0), stop=(blk == BLK - 1))
                for blk in range(BLK):
                    nc.tensor.matmul(ps_q[:, :qw], ones_kr, xsq_c[:, blk, :qw],
                                     start=(blk == 0), stop=(blk == BLK - 1))
                mean_t = scr.tile([128, 512], FP32, tag="lnkeep", bufs=4)
                nc.vector.tensor_scalar(out=mean_t[:, :qw], in0=ps_s[:, :qw],
                                        scalar1=1.0 / C, scalar2=None, op0=ALU.mult)
                m2 = scr.tile([128, 512], FP32, tag="lntmp", bufs=3)
                nc.vector.tensor_tensor(m2[:, :qw], mean_t[:, :qw], mean_t[:, :qw],
                                        ALU.mult)
                var_c = scr.tile([128, 512], FP32, tag="lntmp", bufs=3)
                nc.vector.tensor_scalar(out=var_c[:, :qw], in0=ps_q[:, :qw],
                                        scalar1=1.0 / C, scalar2=None, op0=ALU.mult)
                nc.vector.tensor_tensor(var_c[:, :qw], var_c[:, :qw], m2[:, :qw],
                                        ALU.subtract)
                sd_c = scr.tile([128, 512], FP32, tag="lntmp", bufs=3)
                nc.scalar.activation(out=sd_c[:, :qw], in_=var_c[:, :qw],
                                     func=ACT.Sqrt, bias=eps_sb)
                rstd_t = scr.tile([128, 512], FP32, tag="lnkeep", bufs=4)
                nc.vector.reciprocal(out=rstd_t[:, :qw], in_=sd_c[:, :qw])
                for blk in range(BLK):
                    t = scr.tile([128, 512], FP32, tag="t512")
                    nc.vector.tensor_tensor(t[:, :qw], x_in[:, blk, qo:qo + qw],
                                            mean_t[:, :qw], ALU.subtract)
                    nc.vector.tensor_tensor(t[:, :qw], t[:, :qw], rstd_t[:, :qw],
                                            ALU.mult)
                    nc.vector.tensor_scalar(
                        out=xn_out[:, blk, qo:qo + qw], in0=t[:, :qw],
                        scalar1=lng_sb[:, blk:blk + 1], scalar2=lnb_sb[:, blk:blk + 1],
                        op0=ALU.mult, op1=ALU.add)

        # ---- attention ----
        xn_sb = sb.tile([128, BLK, S], FP32R, name="xn_sb")
        layernorm(x_sb, xn_sb, "ln1")

        qk_sb = sb.tile([128, 4, S], BF16)
        for mt in range(4):
            for (qo, qw) in QC:
                p = psmm.tile([128, 512], FP32, tag="mm")
                for blk in range(BLK):
                    nc.tensor.matmul(p[:, :qw],
                                     wqkT_sb[:, blk, mt * 128:(mt + 1) * 128],
                                     xn_sb[:, blk, qo:qo + qw],
                                     start=(blk == 0), stop=(blk == BLK - 1))
                nc.scalar.activation(out=qk_sb[:, mt, qo:qo + qw], in_=p[:, :qw],
                                     func=ACT.Identity, bias=bqk_sb[:, mt:mt + 1])

        v_sb = sb.tile([128, NKT, HEADS, 65], BF16)
        nc.vector.tensor_copy(
            out=v_sb[:, :, :, 64:65],
            in_=ones_f[:, 0:1, None, None].to_broadcast([128, NKT, HEADS, 1]))
        for kt, (ko, kw) in enumerate(KT):
            p = psmm.tile([128, 512], FP32, tag="mm")
            for blk in range(BLK):
                nc.tensor.matmul(p[:kw, :C], xn_sb[:, blk, ko:ko + kw],
                                 wvT_sb[:, blk, :],
                                 start=(blk == 0), stop=(blk == BLK - 1))
            nc.vector.tensor_tensor(
                v_sb[:kw, kt, :, 0:64],
                p[:kw, :C].rearrange("p (h d) -> p h d", h=HEADS),
                bv_sb[:kw].rearrange("p (h d) -> p h d", h=HEADS),
                ALU.add)

        attn_sb = sb.tile([128, BLK, S], FP32R, tag="attn")
        for (qo, qw) in QC:
            for h in range(HEADS):
                pr = slice((h % 2) * 64, (h % 2) * 64 + 64)
                qblk = h // 2
                exp_c = expp.tile([128, NKT, 512], BF16, tag="exp")
                for kt, (ko, kw) in enumerate(KT):
                    ps_sc = psmm.tile([128, 512], FP32, tag="mm")
                    nc.tensor.matmul(ps_sc[:kw, :qw],
                                     qk_sb[pr, 2 + qblk, ko:ko + kw],
                                     qk_sb[pr, qblk, qo:qo + qw],
                                     start=True, stop=True)
                    nc.scalar.activation(out=exp_c[:kw, kt, :qw], in_=ps_sc[:kw, :qw],
                                         func=ACT.Exp,
                                         bias=kb_sb[:kw, kt:kt + 1], scale=0.125)
                ps_o = pso.tile([65, 512], FP32, tag="o")
                for kt, (ko, kw) in enumerate(KT):
                    nc.tensor.matmul(ps_o[:, :qw], v_sb[:kw, kt, h, :],
                                     exp_c[:kw, kt, :qw],
                                     start=(kt == 0), stop=(kt == NKT - 1))
                rec_f = scr.tile([1, 512], FP32, tag="s512")
                nc.vector.reciprocal(out=rec_f[:, :qw], in_=ps_o[64:65, :qw])
                recd = scr1.tile([1, 512], FP32R, tag="recd")
                nc.vector.tensor_copy(out=recd[:, :qw], in_=rec_f[:, :qw])
                rep_d = psrep.tile([128, 512], FP32, tag="rep")
                nc.tensor.matmul(rep_d[0:64, :qw], ones_m[:, 0:64], recd[:, :qw],
                                 start=True, stop=True)
                onum = scr1.tile([64, 512], FP32, tag="onum")
                nc.scalar.activation(out=onum[:, :qw], in_=ps_o[0:64, :qw],
                                     func=ACT.Copy)
                nc.vector.tensor_tensor(attn_sb[pr, qblk, qo:qo + qw],
                                        onum[:, :qw], rep_d[0:64, :qw], ALU.mult)

        x2_sb = sb.tile([128, BLK, S], FP32)
        for (qo, qw) in QC:
            for mt in range(BLK):
                p = psmm.tile([128, 512], FP32, tag="mm")
                for blk in range(BLK):
                    nc.tensor.matmul(p[:, :qw],
                                     woT_sb[:, blk, mt * 128:(mt + 1) * 128],
                                     attn_sb[:, blk, qo:qo + qw],
                                     start=(blk == 0), stop=(blk == BLK - 1))
                t = scr.tile([128, 512], FP32, tag="t512")
                nc.scalar.activation(out=t[:, :qw], in_=p[:, :qw],
                                     func=ACT.Identity, bias=bo_sb[:, mt:mt + 1])
                nc.vector.tensor_tensor(x2_sb[:, mt, qo:qo + qw], t[:, :qw],
                                        x_sb[:, mt, qo:qo + qw], ALU.add)

        # ---- FFN ----
        xn2_sb = sb.tile([128, BLK, S], FP32R, tag="attn")  # reuse attn buffer
        layernorm(x2_sb, xn2_sb, "ln2")

        y_sb = sb.tile([128, BLK, S], FP32, tag="xy")       # reuse x buffer
        yg_r = yg_d.rearrange("(blk p) t -> p blk t", p=128)
        for (qo, qw) in QC:
            h1_full = expp.tile([128, FB, 512], BF16, tag="h1", name="h1_full")
            h1_c = h1_full
            for mt in range(FB):
                p = psmm.tile([128, 512], FP32, tag="mm")
                for blk in range(BLK):
                    nc.tensor.matmul(p[:, :qw],
                                     w1T_sb[:, blk, mt * 128:(mt + 1) * 128],
                                     xn2_sb[:, blk, qo:qo + qw],
                                     start=(blk == 0), stop=(blk == BLK - 1))
                nc.scalar.activation(out=h1_c[:, mt, :qw], in_=p[:, :qw],
                                     func=ACT.Gelu, bias=b1_sb[:, mt:mt + 1])
            for mt in range(BLK):
                p = psmm.tile([128, 512], FP32, tag="mm")
                for kb in range(FB):
                    nc.tensor.matmul(p[:, :qw],
                                     w2T_sb[:, kb, mt * 128:(mt + 1) * 128],
                                     h1_c[:, kb, :qw],
                                     start=(kb == 0), stop=(kb == FB - 1))
                t = scr.tile([128, 512], FP32, tag="t512")
                nc.scalar.activation(out=t[:, :qw], in_=p[:, :qw],
                                     func=ACT.Identity, bias=b2_sb[:, mt:mt + 1])
                nc.vector.tensor_tensor(y_sb[:, mt, qo:qo + qw], t[:, :qw],
                                        x2_sb[:, mt, qo:qo + qw], ALU.add)
            nc.sync.dma_start(yg_r[:, :, qo:qo + qw], y_sb[:, :, qo:qo + qw])
    return nc


# ----------------------------------------------------------------------------
# Host orchestration
# ----------------------------------------------------------------------------
_CACHE = {}


def _get_agent_nc():
    if "agent" not in _CACHE:
        nc = build_agent_nc()
        nc.finalize()
        _CACHE["agent"] = nc
    return _CACHE["agent"]


def _get_mixer_nc(S):
    key = ("mixer", S)
    if key not in _CACHE:
        nc = build_mixer_nc(S)
        nc.finalize()
        _CACHE[key] = nc
    return _CACHE[key]


def kernel(f_ir, f_vis, aw1, ab1, aw2, ab2,
           ir_lng, ir_lnb, ir_wqkv, ir_bqkv, ir_wo, ir_bo, ir_w1, ir_b1,
           ir_w2, ir_b2,
           vis_lng, vis_lnb, vis_wqkv, vis_bqkv, vis_wo, vis_bo, vis_w1,
           vis_b1, vis_w2, vis_b2):
    f_ir = np.ascontiguousarray(f_ir, np.float32)
    f_vis = np.ascontiguousarray(f_vis, np.float32)

    # ---- launch A: routing logits + base canvas, token-parallel over 8 cores
    fir_n = f_ir.reshape(B, C, N)
    fvis_n = f_vis.reshape(B, C, N)
    X = np.concatenate([fir_n, fvis_n], axis=1)            # [B, 512, N]
    Xf = X.reshape(B, CIN, 2, TPC).transpose(0, 2, 1, 3).reshape(8, CIN, TPC)
    aw1t = np.ascontiguousarray(aw1.T, np.float32)
    aw2t = np.ascontiguousarray(aw2[0], np.float32)
    ab1 = np.ascontiguousarray(ab1, np.float32)
    ab2 = np.ascontiguousarray(ab2, np.float32)

    nc_a = _get_agent_nc()
    in_maps = [dict(x=np.ascontiguousarray(Xf[i]), aw1t=aw1t, ab1=ab1,
                    aw2t=aw2t, ab2=ab2) for i in range(8)]
    ra = run_bass_kernel_spmd(nc_a, in_maps, list(range(8)))
    logits = np.stack([ra.results[i]["logits"] for i in range(8)])
    logits = logits.reshape(B, 2 * TPC)                    # [B, N]
    base = np.stack([ra.results[i]["base"] for i in range(8)])
    base = base.reshape(B, 2, C, TPC).transpose(0, 2, 1, 3).reshape(B, C, N)

    # ---- host routing decision (reference semantics)
    mask = (logits > 0)
    counts = mask.sum(1)
    sel = np.empty_like(mask)
    for b in range(B):
        if counts[b] < MIN_TOK:
            top = np.argsort(-logits[b], kind="stable")[:MIN_TOK]
            s = np.zeros(N, bool)
            s[top] = True
            sel[b] = s
        else:
            sel[b] = mask[b]
    idxs = [np.where(sel[b])[0] for b in range(B)]
    s_max = max(len(i) for i in idxs)
    S = S_PAD_DEFAULT if s_max <= S_PAD_DEFAULT else ((s_max + 63) // 64) * 64
    NKT = len(_chunks(S, 128))

    # ---- launch B: one (batch, modality) mixer per core
    nc_b = _get_mixer_nc(S)
    in_maps_b = []
    metas = []
    for b in range(B):
        idx = idxs[b]
        Sb = len(idx)
        kbias = np.full((128 * NKT,), np.float32(-1e9), np.float32)
        kbias[:Sb] = 0.0
        for mod, fm, pfx in (("ir", fir_n[b], "ir"), ("vis", fvis_n[b], "vis")):
            params = {
                "lng": ir_lng if pfx == "ir" else vis_lng,
                "lnb": ir_lnb if pfx == "ir" else vis_lnb,
                "wqkv": ir_wqkv if pfx == "ir" else vis_wqkv,
                "bqkv": ir_bqkv if pfx == "ir" else vis_bqkv,
                "wo": ir_wo if pfx == "ir" else vis_wo,
                "bo": ir_bo if pfx == "ir" else vis_bo,
                "w1": ir_w1 if pfx == "ir" else vis_w1,
                "b1": ir_b1 if pfx == "ir" else vis_b1,
                "w2": ir_w2 if pfx == "ir" else vis_w2,
                "b2": ir_b2 if pfx == "ir" else vis_b2,
            }
            xg = np.zeros((C, S), np.float32)
            xg[:, :Sb] = fm[:, idx]
            wqkv = np.asarray(params["wqkv"], np.float32)
            im = dict(
                xg=xg, kbias=kbias,
                lng=np.ascontiguousarray(params["lng"], np.float32),
                lnb=np.ascontiguousarray(params["lnb"], np.float32),
                wqkT=np.ascontiguousarray(wqkv[:512].T),
                bqk=np.ascontiguousarray(params["bqkv"][:512], np.float32),
                wvT=np.ascontiguousarray(wqkv[512:].T),
                bv=np.ascontiguousarray(params["bqkv"][512:], np.float32),
                woT=np.ascontiguousarray(np.asarray(params["wo"], np.float32).T),
                bo=np.ascontiguousarray(params["bo"], np.float32),
                w1T=np.ascontiguousarray(np.asarray(params["w1"], np.float32).T),
                b1=np.ascontiguousarray(params["b1"], np.float32),
                w2T=np.ascontiguousarray(np.asarray(params["w2"], np.float32).T),
                b2=np.ascontiguousarray(params["b2"], np.float32),
            )
            in_maps_b.append(im)
            metas.append((b, mod, idx))
    rb = run_bass_kernel_spmd(nc_b, in_maps_b, list(range(8)))

    # ---- host scatter-combine
    out = base  # [B, C, N]; refined overwrites selected positions
    for ci in range(0, 8, 2):
        b, _, idx = metas[ci]
        Sb = len(idx)
        ri = rb.results[ci]["yg"][:, :Sb]
        rv = rb.results[ci + 1]["yg"][:, :Sb]
        refined = (ri + rv) * mask[b, idx].astype(np.float32)[None, :]
        out[b][:, idx] = refined
    return out.reshape(B, C, H, W)
